# revision 39
# baseline (speedup 1.0000x reference)
"""DeltaNet forward kernel for 8 Trainium2 NeuronCores (v3).

Problem (hardcoded): hidden_states [B=4, T=2048, D=1024], H=4 heads, Dh=256,
causal depthwise conv K=4 + silu on q/k/v projections, q/k l2-normalized per
head (q scaled Dh^-0.5), delta-rule recurrence over T, per-head RMSNorm,
merge heads, out = o @ Wo.

Sharding: core c -> batch c//2, head group c%2 (512 projection columns).
Each core computes a partial product against its 512 rows of Wo; the host
sums the two partials per batch.

Design vs baseline:
- q l2norm folded into the output RMSNorm bias:
  out = o_raw / sqrt(mean(o_raw^2) + 256*EPS*|q_raw|^2) (exact up to 2.56e-9).
- Chunked delta rule (C=128) with the chunk inverse computed densely:
  RT = (I+B)^-1 (B = strict upper of K K^T) via 4-level Neumann doubling
  using the transposed-pair trick (track P=B^2^k and P^T together so every
  matmul has its stationary operand pre-transposed). Exponents <= 31;
  validated 1e-4 (f64) / ~3e-3 (fp16) against the exact recurrence.
- Per chunk precompute [Z|W] = R [V|K]; the S-dependent critical path is
  only: pks = W S -> u = Z - pks -> S += K^T u -> copy S (4 hops).
- Both heads interleaved per chunk; head-paired elementwise ops in the
  R chain; phase A runs in 2 halves with half 1 spliced between chunks
  0..7; the output projection streams per 128-token chunk.
- fp16 everywhere (fp8 tested: quantization error does not average down
  for random-sign dot products -> ~4% output error, over budget).
- Activation-table discipline: Copy/Square are in every act table; Silu
  and Sqrt never share one. All Silus batched so tables load ~4x total.
"""

import numpy as np

B, T, D = 4, 2048, 1024
H = 4
DH = D // H          # 256
CONV_K = 4
EPS = 1e-5
NCORES = 8
CG = 512             # columns per core (2 heads)
C = 128              # recurrence chunk length
NCHUNK = T // C      # 16
PAD = 4              # leading zero pad for causal conv
TOKB = 512           # projection token block (psum width)
HALF = 1024          # conv/norm granularity
NLVL = 4             # doubling levels (exponents <= 2^(NLVL+1)-1 = 31)
KT = 8               # contraction tiles for projections
QBS = float(EPS * DH)   # 2.56e-3: q-sumsq scale folded into RMS bias

_CACHE = {}
DBG = False

# tap0 engine per (ti, ct) flat index 0..11: 1 = Act (Copy*scale), 0 = DVE
CONV_ENG = [1] * 12


def _build_bass():
    import concourse.bass as bass  # noqa: F401
    import concourse.bacc as bacc
    import concourse.mybir as mybir
    import concourse.tile as tile

    dt = mybir.dt
    nc = bacc.Bacc("TRN2", target_bir_lowering=False, debug=False)

    xT = nc.dram_tensor("xT", [D, T], dt.float16, kind="ExternalInput")
    wq = nc.dram_tensor("wq", [D, CG], dt.float16, kind="ExternalInput")
    wk = nc.dram_tensor("wk", [D, CG], dt.float16, kind="ExternalInput")
    wv = nc.dram_tensor("wv", [D, CG], dt.float16, kind="ExternalInput")
    wo = nc.dram_tensor("wo", [CG, D], dt.float16, kind="ExternalInput")
    cw = nc.dram_tensor("cw", [4, 128, 3 * CONV_K], dt.float32,
                        kind="ExternalInput")
    consts = nc.dram_tensor("consts", [128, 1152], dt.float16,
                            kind="ExternalInput")
    out = nc.dram_tensor("out", [T, D], dt.float16, kind="ExternalOutput")
    dbg = nc.dram_tensor("dbg", [128, 5120], dt.float32,
                         kind="ExternalOutput") if DBG else None

    with tile.TileContext(nc) as tc:
        _body(nc, tc, mybir, xT, wq, wk, wv, wo, cw, consts, out, dbg)

    nc.compile()
    return nc


def _body(nc, tc, mybir, xT, wq, wk, wv, wo, cw, consts, out, dbg=None):
    dt = mybir.dt
    AF = mybir.ActivationFunctionType
    ALU = mybir.AluOpType
    fp32 = dt.float32
    f16 = dt.float16

    xT_t = xT.ap().rearrange("(n p) t -> n p t", p=128)      # [8,128,T]
    w_t = {"q": wq.ap().rearrange("(n p) c -> n p c", p=128),
           "k": wk.ap().rearrange("(n p) c -> n p c", p=128),
           "v": wv.ap().rearrange("(n p) c -> n p c", p=128)}
    wo_t = wo.ap().rearrange("(n p) c -> n p c", p=128)      # [4,128,D]
    cw_t = cw.ap()                                           # [4,128,12]
    out_t = out.ap().rearrange("(n p) d -> n p d", p=128)    # [16,128,D]

    bw = [None]   # bwork pool, created after xwp release

    with tc.tile_pool(name="persist", bufs=1) as persist, \
         tc.tile_pool(name="qkvp", bufs=1) as qkvp, \
         tc.tile_pool(name="rawp", bufs=1) as rawp, \
         tc.tile_pool(name="sqp", bufs=1) as sqp, \
         tc.tile_pool(name="normp", bufs=2) as normp, \
         tc.tile_pool(name="ofp", bufs=3) as ofp, \
         tc.tile_pool(name="bigps", bufs=2, space="PSUM") as bigps, \
         tc.tile_pool(name="rps", bufs=2, space="PSUM") as rps, \
         tc.tile_pool(name="kps", bufs=2, space="PSUM") as kps, \
         tc.tile_pool(name="tps", bufs=2, space="PSUM") as tps:

        # ---------------- loads ----------------
        xwp = tc.alloc_tile_pool(name="xwp", bufs=1)
        cons = persist.tile([128, 1152], f16, name="cons", tag="cons")
        nc.sync.dma_start(cons[:], consts.ap())
        ident = cons[:, 0:128]        # I
        m_su2 = cons[:, 128:384]      # [+1 a<b] twice (head-pair masks)
        m_sl2 = cons[:, 384:640]      # [+1 a>b] twice
        m_R02 = cons[:, 640:896]      # [I - strict-upper] twice
        m_tri2 = cons[:, 896:1152]    # [+1 a<=b] twice
        ones_col = cons[:, 1023:1024]  # last col of triuI mask == all ones

        bias6 = persist.tile([128, 1], fp32, name="bias6", tag="bias6")
        nc.vector.memset(bias6[:], 1e-6)

        cwt = []
        for ct in range(4):
            t_ = persist.tile([128, 3 * CONV_K], fp32, name=f"cw{ct}",
                              tag=f"cw{ct}")
            nc.sync.dma_start(t_[:], cw_t[ct])
            cwt.append(t_)

        xt = []
        for kt in range(KT):
            t_ = xwp.tile([128, T], f16, name=f"xt{kt}", tag=f"xt{kt}")
            nc.sync.dma_start(t_[:], xT_t[kt])
            xt.append(t_)
        ws = {}
        for nm in ("q", "k", "v"):
            ws[nm] = []
            for kt in range(KT):
                t_ = xwp.tile([128, CG], f16, name=f"w{nm}{kt}",
                              tag=f"w{nm}{kt}")
                nc.sync.dma_start(t_[:], w_t[nm][kt])
                ws[nm].append(t_)
        wlist = [ws["q"], ws["k"], ws["v"]]
        wo_s = []
        for ct in range(4):
            t_ = persist.tile([128, D], f16, name=f"wos{ct}", tag=f"wos{ct}")
            nc.sync.dma_start(t_[:], wo_t[ct])
            wo_s.append(t_)

        # ---------------- persistent working tensors ----------------
        # qkh[ct]: [q | k] over time; vh[ct]: v; oTp[h]: output^T pair layout
        qkh = [qkvp.tile([128, 2 * T], f16, name=f"qkh{ct}", tag=f"qkh{ct}")
               for ct in range(4)]
        vh = [qkvp.tile([128, T], f16, name=f"vh{ct}", tag=f"vh{ct}")
              for ct in range(4)]
        oTp = [qkvp.tile([128, 2 * T], f16, name=f"oTp{h}", tag=f"oTp{h}")
               for h in range(2)]
        raw = [rawp.tile([128, HALF + PAD], f16, name=f"raw{i}", tag=f"raw{i}")
               for i in range(12)]
        for i in range(12):
            nc.gpsimd.memset(raw[i][:, 0:PAD], 0.0)

        s_sb = [None, None]

        # diag(conv weight) tiles for the v-projection conv-as-matmul
        dgv = []
        for ct in range(4):
            row = []
            for i in range(CONV_K):
                d_ = persist.tile([128, 128], f16, name=f"dgv{ct}{i}",
                                  tag=f"dgv{ct}{i}")
                nc.vector.tensor_scalar_mul(
                    d_[:], ident, cwt[ct][:, 2 * CONV_K + i:2 * CONV_K + i + 1])
                row.append(d_)
            dgv.append(row)

        # ============ phase A emission (per half) ============
        def emit_proj_block(half, nb):
            """Projection matmuls + psum->raw copies for one 512-token block."""
            gb = 2 * half + nb
            for ti in range(3):
                for ct in range(4):
                    idx = ti * 4 + ct
                    pp = bigps.tile([128, TOKB], fp32, name=f"pp{gb}{idx}",
                                    tag="big")
                    for kt in range(KT):
                        nc.tensor.matmul(
                            pp[:], wlist[ti][kt][:, ct * 128:(ct + 1) * 128],
                            xt[kt][:, gb * TOKB:(gb + 1) * TOKB],
                            start=(kt == 0), stop=(kt == KT - 1))
                    dst = raw[idx][:, PAD + nb * TOKB:PAD + (nb + 1) * TOKB]
                    if idx % 2 == 0:
                        nc.scalar.copy(dst, pp[:])
                    else:
                        nc.vector.tensor_copy(dst, pp[:])

        def _conv_dst(half, ti, ct):
            t0 = half * HALF
            if ti == 0:
                return qkh[ct][:, t0:t0 + HALF]
            if ti == 1:
                return qkh[ct][:, T + t0:T + t0 + HALF]
            return vh[ct][:, t0:t0 + HALF]

        def emit_conv_taps(half, ti, ct):
            """Causal conv (4 taps) for one (proj, ct) over one half.
            Silu is emitted separately to batch activation-table usage.
            v tiles (ti==2) run the conv on the PE as accumulating
            diag-weight matmuls, with Silu consuming the psum directly."""
            idx = ti * 4 + ct
            dst = _conv_dst(half, ti, ct)
            if ti == 2:
                for nb in range(2):
                    cv = bigps.tile([128, TOKB], fp32, name=f"cv{half}{ct}{nb}",
                                    tag="big")
                    for i in range(CONV_K):
                        nc.tensor.matmul(
                            cv[:], dgv[ct][i],
                            raw[idx][:, 1 + i + nb * TOKB:
                                     1 + i + nb * TOKB + TOKB],
                            start=(i == 0), stop=(i == CONV_K - 1))
                    nc.scalar.activation(
                        dst[:, nb * TOKB:(nb + 1) * TOKB], cv[:], AF.Silu)
                if half == 0:
                    nc.gpsimd.tensor_copy(raw[idx][:, 0:PAD],
                                          raw[idx][:, HALF:HALF + PAD])
                return
            w0 = cwt[ct][:, ti * CONV_K:ti * CONV_K + 1]
            nc.scalar.activation(dst, raw[idx][:, 1:1 + HALF], AF.Copy,
                                 scale=w0)
            tta = sqp.tile([128, HALF], f16, name=f"cta{half}{idx}", tag="cta",
                           bufs=3)
            ttb = sqp.tile([128, HALF], f16, name=f"ctb{half}{idx}", tag="ctb",
                           bufs=3)
            w1 = cwt[ct][:, ti * CONV_K + 1:ti * CONV_K + 2]
            w2 = cwt[ct][:, ti * CONV_K + 2:ti * CONV_K + 3]
            w3 = cwt[ct][:, ti * CONV_K + 3:ti * CONV_K + 4]
            nc.vector.tensor_scalar_mul(tta[:], raw[idx][:, 2:2 + HALF], w1)
            nc.vector.tensor_scalar_mul(ttb[:], raw[idx][:, 3:3 + HALF], w2)
            nc.vector.tensor_add(tta[:], tta[:], ttb[:])
            nc.vector.tensor_scalar_mul(ttb[:], raw[idx][:, 4:4 + HALF], w3)
            nc.vector.tensor_add(dst, dst, tta[:])
            nc.vector.tensor_add(dst, dst, ttb[:])
            # boundary carry for next half (tokens 1020..1023 -> cols 0..3)
            if half == 0:
                nc.gpsimd.tensor_copy(raw[idx][:, 0:PAD],
                                      raw[idx][:, HALF:HALF + PAD])

        def emit_silu(half, ti, ct):
            if ti == 2:
                return
            dst = _conv_dst(half, ti, ct)
            nc.scalar.activation(dst, dst, AF.Silu)

        sq_q = {}   # (half, ct) -> [128, HALF] q^2 tiles for the RMS bias
        def emit_norms(half):
            """k l2norm (+ sq_q tiles) for one half."""
            t0 = half * HALF
            etn = nc.gpsimd if half == 0 else nc.vector
            for ct in range(4):
                t_ = sqp.tile([128, HALF], f16, name=f"sqq{half}{ct}",
                              tag=f"sqq{ct}", bufs=2)
                qs = qkh[ct][:, t0:t0 + HALF]
                etn.tensor_mul(t_[:], qs, qs)
                sq_q[(half, ct)] = t_
            for head in range(2):
                sqk = []
                for i in range(2):
                    ct = 2 * head + i
                    t_ = sqp.tile([128, HALF], f16, name=f"sqk{half}{ct}",
                                  tag="cta", bufs=3)
                    ks = qkh[ct][:, T + t0:T + t0 + HALF]
                    etn.tensor_mul(t_[:], ks, ks)
                    sqk.append(t_)
                bcf = normp.tile([128, HALF], fp32, name=f"bcf{half}{head}",
                                 tag="bcf", bufs=1)
                for nb in range(2):
                    prow = bigps.tile([1, TOKB], fp32,
                                      name=f"pr{half}{head}{nb}", tag="big")
                    for i in range(2):
                        nc.tensor.matmul(prow[:], ones_col,
                                         sqk[i][:, nb * TOKB:(nb + 1) * TOKB],
                                         start=(i == 0), stop=(i == 1))
                    rowb = normp.tile([1, TOKB], fp32,
                                      name=f"rb{half}{head}{nb}", tag="rowb",
                                      bufs=3)
                    nc.scalar.copy(rowb[:], prow[:])
                    nc.gpsimd.partition_broadcast(
                        bcf[:, nb * TOKB:(nb + 1) * TOKB], rowb[:])
                nc.scalar.activation(bcf[:], bcf[:], AF.Sqrt,
                                     bias=bias6[:, 0:1])
                nc.vector.reciprocal(bcf[:], bcf[:])
                bcb = normp.tile([128, HALF], f16, name=f"bcb{half}{head}",
                                 tag="bcb")
                etn.tensor_copy(bcb[:], bcf[:])
                for i in range(2):
                    ct = 2 * head + i
                    ks = qkh[ct][:, T + t0:T + t0 + HALF]
                    etn.tensor_mul(ks, ks, bcb[:])

        # ============ phase B emission: software-pipelined stages ============
        # PSUM rings (bank-granular, 8 banks):
        #   bigps x2: pp/prow (phase A), zw, pf
        #   rps  x2: rp [P2 pair | PT2 pair], dac [acc pair]
        #   kps  x2: qps, pkkq, pks, ksu0, ksu1, po
        #   tps  x2: kvt (f16 x4), wot (WT + oT, f16 x4)
        # Iteration k emits chunk k's precompute (R doubling etc.) with chunk
        # k-1's chain/output stages spliced between the R levels, so every
        # engine has ready work queued during the R ping-pong latencies.
        ST = {}

        def st_pre(ch):
            t0 = ch * C
            half = ch // 8
            st = ST[ch] = {}
            kvt = tps.tile([128, 1024], f16, name=f"kvt{ch}", tag="tps")
            qps_t = kps.tile([128, 2], fp32, name=f"qps{ch}", tag="kps")
            pkkq = kps.tile([128, 512], fp32, name=f"pkkq{ch}", tag="kps")
            rp = rps.tile([128, 512], fp32, name=f"rp{ch}", tag="rps")
            Bp = bw[0].tile([128, 256], f16, name=f"Bp{ch}", tag="Bp")
            Ap = bw[0].tile([128, 256], f16, name=f"Ap{ch}", tag="Ap")
            R0p = bw[0].tile([128, 256], f16, name=f"R0p{ch}", tag="Rp", bufs=4)
            rhs_kv = [None, None]
            Pat = [None, None]
            for h in range(2):
                ct0 = 2 * h
                for srcv in range(2):  # 0: v, 1: k
                    for i in range(2):
                        if srcv == 0:
                            ap = vh[ct0 + i][:, t0:t0 + C]
                        else:
                            ap = qkh[ct0 + i][:, T + t0:T + t0 + C]
                        o0 = 512 * h + 256 * srcv + 128 * i
                        nc.tensor.transpose(kvt[:, o0:o0 + 128], ap, ident)
                rkv = bw[0].tile([128, 512], f16, name=f"rkv{ch}{h}", tag="rkv",
                                 bufs=4)
                nc.scalar.copy(rkv[:], kvt[:, 512 * h:512 * (h + 1)])
                rhs_kv[h] = rkv
                pk = pkkq[:, 256 * h:256 * (h + 1)]
                for i in range(2):
                    qk2 = qkh[ct0 + i].rearrange(
                        "p (n t) -> p n t", n=2)[:, :, t0:t0 + C]
                    nc.tensor.matmul(pk, qkh[ct0 + i][:, T + t0:T + t0 + C],
                                     qk2, start=(i == 0), stop=(i == 1))
                qps = qps_t[:, h:h + 1]
                for i in range(2):
                    nc.tensor.matmul(qps, sq_q[(half, ct0 + i)][
                        :, t0 - half * HALF:t0 - half * HALF + C],
                        ones_col, start=(h == 0 and i == 0), stop=(i == 1),
                        skip_group_check=True)
            qbp = bw[0].tile([128, 2], fp32, name=f"qb{ch}", tag="qb", bufs=4)
            nc.scalar.activation(qbp[:], qps_t[:], AF.Copy, scale=QBS)
            # head-paired mask ops ([h0|h1] strided reads of pkkq)
            pkk2 = pkkq.rearrange("p (h c) -> p h c", h=2)[:, :, 128:256]
            pkq2 = pkkq.rearrange("p (h c) -> p h c", h=2)[:, :, 0:128]
            B2 = Bp.rearrange("p (h c) -> p h c", h=2)
            A2_ = Ap.rearrange("p (h c) -> p h c", h=2)
            M2 = m_su2.rearrange("p (h c) -> p h c", h=2)
            nc.vector.tensor_mul(B2, pkk2, M2)
            nc.vector.tensor_mul(A2_, pkk2,
                                 m_sl2.rearrange("p (h c) -> p h c", h=2))
            for h in range(2):
                hs = slice(128 * h, 128 * (h + 1))
                nc.vector.tensor_sub(R0p[:, hs], ident, Bp[:, hs])
            Patp = bw[0].tile([128, 256], f16, name=f"Pat{ch}", tag="Pat",
                              bufs=4)
            nc.vector.tensor_mul(Patp.rearrange("p (h c) -> p h c", h=2),
                                 pkq2, m_tri2.rearrange("p (h c) -> p h c", h=2))
            Pat = [Patp[:, 0:128], Patp[:, 128:256]]
            st.update(rhs_kv=rhs_kv, Pat=Pat, qb=[qbp[:, 0:1], qbp[:, 1:2]],
                      rp=rp, RT=R0p, Pm=Bp, PTm=Ap)

        def st_rlvl(ch, lvl):
            st = ST[ch]
            rp, RT, Pm, PTm = st["rp"], st["RT"], st["Pm"], st["PTm"]
            for h in range(2):
                hs = slice(128 * h, 128 * (h + 1))
                if lvl < NLVL - 1:
                    nc.tensor.matmul(rp[:, hs], PTm[:, hs], Pm[:, hs],
                                     start=True, stop=True,
                                     skip_group_check=True)
                nc.tensor.matmul(rp[:, 256 + 128 * h:256 + 128 * (h + 1)],
                                 Pm[:, hs], PTm[:, hs], start=True,
                                 stop=True, skip_group_check=True)
            PTn = bw[0].tile([128, 256], f16, name=f"ptn{ch}{lvl}", tag="PT",
                             bufs=4)
            nc.vector.tensor_copy(PTn[:], rp[:, 256:512])
            if lvl < NLVL - 1:
                Pn = bw[0].tile([128, 256], f16, name=f"pn{ch}{lvl}", tag="P",
                                bufs=4)
                nc.scalar.copy(Pn[:], rp[:, 0:256])
            else:
                Pn = None
            for h in range(2):
                hs = slice(128 * h, 128 * (h + 1))
                nc.tensor.matmul(rp[:, hs], PTn[:, hs], RT[:, hs],
                                 start=True, stop=True, skip_group_check=True)
            RTn = bw[0].tile([128, 256], f16, name=f"rt{ch}{lvl}", tag="Rp",
                             bufs=4)
            nc.vector.tensor_add(RTn[:], RT[:], rp[:, 0:256])
            st.update(RT=RTn, Pm=Pn, PTm=PTn)

        def st_zw(ch):
            st = ST[ch]
            RT, rhs_kv = st["RT"], st["rhs_kv"]
            zwp = bw[0].tile([128, 1024], f16, name=f"zwp{ch}", tag="zw")
            wtp = tps.tile([128, 512], f16, name=f"wtp{ch}", tag="tps")
            for h in range(2):
                zw = bigps.tile([128, 512], fp32, name=f"zw{ch}{h}", tag="big")
                nc.tensor.matmul(zw[:], RT[:, 128 * h:128 * (h + 1)],
                                 rhs_kv[h][:], start=True, stop=True)
                if h == 0:
                    nc.scalar.copy(zwp[:, 0:512], zw[:])
                else:
                    nc.scalar.copy(zwp[:, 512:1024], zw[:])
            for h in range(2):
                for i in range(2):
                    nc.tensor.transpose(
                        wtp[:, 256 * h + 128 * i:256 * h + 128 * (i + 1)],
                        zwp[:, 512 * h + 256 + 128 * i:
                            512 * h + 256 + 128 * (i + 1)],
                        ident)
            wts = bw[0].tile([128, 512], f16, name=f"wts{ch}", tag="wt")
            nc.scalar.copy(wts[:], wtp[:])
            st.update(zwp=zwp, wts=wts)

        def st_chain1(ch):
            st = ST[ch]
            zwp, wts = st["zwp"], st["wts"]
            s_prev = [s_sb[0], s_sb[1]]
            up = bw[0].tile([128, 512], f16, name=f"up{ch}", tag="u", bufs=4)
            zsel = zwp.rearrange("p (n c) -> p n c", n=4)[:, 0::2, :]
            if ch == 0:
                nc.vector.tensor_copy(
                    up.rearrange("p (n c) -> p n c", n=2), zsel)
            else:
                pks_t = kps.tile([128, 512], fp32, name=f"pks{ch}", tag="kps")
                for h in range(2):
                    pks = pks_t[:, 256 * h:256 * (h + 1)]
                    for i in range(2):
                        nc.tensor.matmul(
                            pks,
                            wts[:, 256 * h + 128 * i:256 * h + 128 * (i + 1)],
                            s_prev[h][:, i * 256:(i + 1) * 256],
                            start=(i == 0), stop=(i == 1))
                nc.vector.tensor_sub(
                    up.rearrange("p (n c) -> p n c", n=2), zsel, pks_t[:])
            st.update(up=up, s_prev=s_prev)

        def st_chain2(ch):
            st = ST[ch]
            up, s_prev, rhs_kv = st["up"], st["s_prev"], st["rhs_kv"]
            for h in range(2):
                ksu = kps.tile([128, 512], fp32, name=f"ksu{ch}{h}", tag="kps")
                for i in range(2):
                    # start once per bank: start=True marks the WHOLE 2KB
                    # bank pending-zero; i=1's start=False write overwrites
                    # its still-pending half (init semantics).
                    nc.tensor.matmul(
                        ksu[:, i * 256:(i + 1) * 256],
                        rhs_kv[h][:, 256 + 128 * i:256 + 128 * (i + 1)],
                        up[:, 256 * h:256 * (h + 1)],
                        start=(i == 0), stop=True, skip_group_check=True)
                s_n = bw[0].tile([128, 512], f16, name=f"ssb{ch}{h}",
                                 tag="ssb", bufs=4)
                if ch == 0:
                    if h == 0:
                        nc.vector.tensor_copy(s_n[:], ksu[:])
                    else:
                        nc.scalar.copy(s_n[:], ksu[:])
                else:
                    nc.vector.tensor_add(s_n[:], s_prev[h][:], ksu[:])
                s_sb[h] = s_n

        def st_o1(ch):
            st = ST[ch]
            t0 = ch * C
            up, s_prev, Pat, qb = st["up"], st["s_prev"], st["Pat"], st["qb"]
            po_t = rps.tile([128, 512], fp32, name=f"po{ch}", tag="rps")
            onrm = bw[0].tile([128, 512], f16, name=f"onrm{ch}", tag="onrm")
            for h in range(2):
                ct0 = 2 * h
                po = po_t[:, 256 * h:256 * (h + 1)]
                if ch == 0:
                    nc.tensor.matmul(po, Pat[h],
                                     up[:, 256 * h:256 * (h + 1)],
                                     start=(h == 0), stop=True,
                                     skip_group_check=True)
                else:
                    for i in range(2):
                        nc.tensor.matmul(po, qkh[ct0 + i][:, t0:t0 + C],
                                         s_prev[h][:, i * 256:(i + 1) * 256],
                                         start=(h == 0 and i == 0), stop=False,
                                         skip_group_check=True)
                    nc.tensor.matmul(po, Pat[h],
                                     up[:, 256 * h:256 * (h + 1)],
                                     start=False, stop=True,
                                     skip_group_check=True)
                osq = bw[0].tile([128, 256], f16, name=f"osq{ch}{h}",
                                 tag="osq")
                ossq = bw[0].tile([128, 1], fp32, name=f"ossq{ch}{h}",
                                  tag="ossq", bufs=4)
                nc.scalar.activation(osq[:], po, AF.Square, accum_out=ossq[:])
                orsq = bw[0].tile([128, 1], fp32, name=f"orsq{ch}{h}",
                                  tag="orsq", bufs=4)
                nc.scalar.activation(orsq[:], ossq[:], AF.Sqrt,
                                     bias=qb[h], scale=1.0 / DH)
                nc.vector.reciprocal(orsq[:], orsq[:])
                nc.vector.tensor_scalar_mul(onrm[:, 256 * h:256 * (h + 1)],
                                            po, orsq[:])
            st.update(onrm=onrm)

        def st_o2c(ch):
            st = ST[ch]
            t0 = ch * C
            onrm = st["onrm"]
            otp = tps.tile([128, 512], f16, name=f"otp{ch}", tag="tps")
            for h in range(2):
                for i in range(2):
                    nc.tensor.transpose(
                        otp[:, 256 * h + 128 * i:256 * h + 128 * (i + 1)],
                        onrm[:, 256 * h + i * 128:256 * h + (i + 1) * 128],
                        ident)
                dstp = oTp[h].rearrange("p (n t) -> p n t",
                                        n=2)[:, :, t0:t0 + C]
                srcp = otp[:, 256 * h:256 * (h + 1)].rearrange(
                    "p (n t) -> p n t", n=2)
                nc.scalar.activation(dstp, srcp, AF.Copy)
            for hf in range(2):
                pf = bigps.tile([128, 512], fp32, name=f"pf{ch}{hf}",
                                tag="big")
                for ct in range(4):
                    h, i = divmod(ct, 2)
                    nc.tensor.matmul(
                        pf[:], oTp[h][:, i * T + t0:i * T + t0 + C],
                        wo_s[ct][:, hf * 512:(hf + 1) * 512],
                        start=(ct == 0), stop=(ct == 3))
                of = ofp.tile([128, 512], f16, name=f"of{ch}{hf}", tag="of")
                nc.scalar.copy(of[:], pf[:])
                nc.sync.dma_start(out_t[ch][:, hf * 512:(hf + 1) * 512], of[:])
            del ST[ch]

        # ============ top-level emission order ============
        emit_proj_block(0, 0)
        emit_proj_block(0, 1)
        for ti in range(3):
            for ct in range(4):
                emit_conv_taps(0, ti, ct)
                emit_silu(0, ti, ct)
        emit_norms(0)
        emit_proj_block(1, 0)
        emit_proj_block(1, 1)
        # projection inputs are dead now; reuse their SBUF for phase B work
        xwp.release()
        bw[0] = tc.alloc_tile_pool(name="bwork", bufs=3)
        for h in range(2):
            t_ = bw[0].tile([128, 512], f16, name=f"ssb{h}_init", tag="ssb",
                            bufs=4)
            nc.vector.memset(t_[:], 0.0)
            s_sb[h] = t_

        def _silus_norms():
            for ti in range(3):
                for ct in range(4):
                    emit_silu(1, ti, ct)
            emit_norms(1)

        a1 = [
            lambda: [emit_conv_taps(1, 0, ct) for ct in range(4)],
            lambda: [emit_conv_taps(1, 1, ct) for ct in range(4)],
            lambda: [emit_conv_taps(1, 2, ct) for ct in range(4)],
            _silus_norms,
        ]

        for it in range(NCHUNK // 2 + 1):
            c0, c1 = 2 * it, 2 * it + 1
            p0, p1 = c0 - 2, c1 - 2
            pre = c0 < NCHUNK
            if pre:
                st_pre(c0)
                st_pre(c1)
                st_rlvl(c0, 0)
                st_rlvl(c1, 0)
            if p0 >= 0:
                st_chain1(p0)
            if pre:
                st_rlvl(c0, 1)
                st_rlvl(c1, 1)
            if p0 >= 0:
                st_chain2(p0)
                st_chain1(p1)
            if pre:
                st_rlvl(c0, 2)
                st_rlvl(c1, 2)
            if p0 >= 0:
                st_chain2(p1)
                st_o1(p0)
            if pre:
                st_rlvl(c0, 3)
                st_rlvl(c1, 3)
            if p0 >= 0:
                st_o1(p1)
                st_o2c(p0)
                st_o2c(p1)
            if pre:
                st_zw(c0)
                st_zw(c1)
            if it < len(a1):
                a1[it]()
        bw[0].release()


LP_NP = np.float16


def _make_consts():
    ii = np.arange(128)
    ident = np.eye(128, dtype=np.float32)
    m_su = (ii[:, None] < ii[None, :]).astype(np.float32)
    m_sl = (ii[:, None] > ii[None, :]).astype(np.float32)
    m_R0 = ident - m_su
    m_triuI = (ii[:, None] <= ii[None, :]).astype(np.float32)
    return np.concatenate([ident, m_su, m_su, m_sl, m_sl, m_R0, m_R0,
                           m_triuI, m_triuI], axis=1).astype(LP_NP)


def _get_compiled():
    if "nc" not in _CACHE:
        _CACHE["nc"] = _build_bass()
    return _CACHE["nc"]


def kernel(hidden_states, Wq, Wk, Wv, conv_wq, conv_wk, conv_wv, onorm_w, Wo):
    from concourse.bass_utils import run_bass_kernel_spmd

    hidden_states = np.asarray(hidden_states, np.float32)
    Wq = np.asarray(Wq, np.float32)
    Wk = np.asarray(Wk, np.float32)
    Wv = np.asarray(Wv, np.float32)
    Wo = np.asarray(Wo, np.float32)
    conv_wq = np.asarray(conv_wq, np.float32)
    conv_wk = np.asarray(conv_wk, np.float32)
    conv_wv = np.asarray(conv_wv, np.float32)
    onorm_w = np.asarray(onorm_w, np.float32)

    consts = _make_consts()
    Wo_eff = (Wo * np.tile(onorm_w, H)[:, None]).astype(LP_NP)

    in_maps = []
    for core in range(NCORES):
        b, g = divmod(core, 2)
        cols = slice(CG * g, CG * (g + 1))
        cwf = np.concatenate([conv_wq[cols], conv_wk[cols], conv_wv[cols]],
                             axis=1)
        in_maps.append({
            "xT": np.ascontiguousarray(hidden_states[b].T).astype(LP_NP),
            "wq": np.ascontiguousarray(Wq[:, cols]).astype(LP_NP),
            "wk": np.ascontiguousarray(Wk[:, cols]).astype(LP_NP),
            "wv": np.ascontiguousarray(Wv[:, cols]).astype(LP_NP),
            "wo": np.ascontiguousarray(Wo_eff[cols, :]),
            "cw": np.ascontiguousarray(cwf.reshape(4, 128, 3 * CONV_K)),
            "consts": consts,
        })

    nc = _get_compiled()
    res = run_bass_kernel_spmd(nc, in_maps, core_ids=list(range(NCORES)),
                               **_CACHE.get("run_kwargs", {}))
    _CACHE["last_results"] = res
    out = np.zeros((B, T, D), np.float32)
    for core in range(NCORES):
        out[core // 2] += res.results[core]["out"].astype(np.float32)
    return out


# revision 41
# speedup vs baseline: 1.0245x; 1.0245x over previous
"""DeltaNet forward kernel for 8 Trainium2 NeuronCores (v3).

Problem (hardcoded): hidden_states [B=4, T=2048, D=1024], H=4 heads, Dh=256,
causal depthwise conv K=4 + silu on q/k/v projections, q/k l2-normalized per
head (q scaled Dh^-0.5), delta-rule recurrence over T, per-head RMSNorm,
merge heads, out = o @ Wo.

Sharding: core c -> batch c//2, head group c%2 (512 projection columns).
Each core computes a partial product against its 512 rows of Wo; the host
sums the two partials per batch.

Design vs baseline:
- q l2norm folded into the output RMSNorm bias:
  out = o_raw / sqrt(mean(o_raw^2) + 256*EPS*|q_raw|^2) (exact up to 2.56e-9).
- Chunked delta rule (C=128) with the chunk inverse computed densely:
  RT = (I+B)^-1 (B = strict upper of K K^T) via 4-level Neumann doubling
  using the transposed-pair trick (track P=B^2^k and P^T together so every
  matmul has its stationary operand pre-transposed). Exponents <= 31;
  validated 1e-4 (f64) / ~3e-3 (fp16) against the exact recurrence.
- Per chunk precompute [Z|W] = R [V|K]; the S-dependent critical path is
  only: pks = W S -> u = Z - pks -> S += K^T u -> copy S (4 hops).
- Both heads interleaved per chunk; head-paired elementwise ops in the
  R chain; phase A runs in 2 halves with half 1 spliced between chunks
  0..7; the output projection streams per 128-token chunk.
- fp16 everywhere (fp8 tested: quantization error does not average down
  for random-sign dot products -> ~4% output error, over budget).
- Activation-table discipline: Copy/Square are in every act table; Silu
  and Sqrt never share one. All Silus batched so tables load ~4x total.
"""

import numpy as np

B, T, D = 4, 2048, 1024
H = 4
DH = D // H          # 256
CONV_K = 4
EPS = 1e-5
NCORES = 8
CG = 512             # columns per core (2 heads)
C = 128              # recurrence chunk length
NCHUNK = T // C      # 16
PAD = 4              # leading zero pad for causal conv
TOKB = 512           # projection token block (psum width)
HALF = 1024          # conv/norm granularity
NLVL = 4             # doubling levels (exponents <= 2^(NLVL+1)-1 = 31)
KT = 8               # contraction tiles for projections
QBS = float(EPS * DH)   # 2.56e-3: q-sumsq scale folded into RMS bias

_CACHE = {}
DBG = False

# tap0 engine per (ti, ct) flat index 0..11: 1 = Act (Copy*scale), 0 = DVE
CONV_ENG = [1] * 12


def _build_bass():
    import concourse.bass as bass  # noqa: F401
    import concourse.bacc as bacc
    import concourse.mybir as mybir
    import concourse.tile as tile

    dt = mybir.dt
    nc = bacc.Bacc("TRN2", target_bir_lowering=False, debug=False)

    xT = nc.dram_tensor("xT", [D, T], dt.float16, kind="ExternalInput")
    wq = nc.dram_tensor("wq", [D, CG], dt.float16, kind="ExternalInput")
    wk = nc.dram_tensor("wk", [D, CG], dt.float16, kind="ExternalInput")
    wv = nc.dram_tensor("wv", [D, CG], dt.float16, kind="ExternalInput")
    wo = nc.dram_tensor("wo", [CG, D], dt.float16, kind="ExternalInput")
    cw = nc.dram_tensor("cw", [4, 128, 3 * CONV_K], dt.float32,
                        kind="ExternalInput")
    consts = nc.dram_tensor("consts", [128, 1152], dt.float16,
                            kind="ExternalInput")
    out = nc.dram_tensor("out", [T, D], dt.float16, kind="ExternalOutput")
    dbg = nc.dram_tensor("dbg", [128, 5120], dt.float32,
                         kind="ExternalOutput") if DBG else None

    with tile.TileContext(nc) as tc:
        _body(nc, tc, mybir, xT, wq, wk, wv, wo, cw, consts, out, dbg)

    nc.compile()
    return nc


def _body(nc, tc, mybir, xT, wq, wk, wv, wo, cw, consts, out, dbg=None):
    dt = mybir.dt
    AF = mybir.ActivationFunctionType
    ALU = mybir.AluOpType
    fp32 = dt.float32
    f16 = dt.float16

    xT_t = xT.ap().rearrange("(n p) t -> n p t", p=128)      # [8,128,T]
    w_t = {"q": wq.ap().rearrange("(n p) c -> n p c", p=128),
           "k": wk.ap().rearrange("(n p) c -> n p c", p=128),
           "v": wv.ap().rearrange("(n p) c -> n p c", p=128)}
    wo_t = wo.ap().rearrange("(n p) c -> n p c", p=128)      # [4,128,D]
    cw_t = cw.ap()                                           # [4,128,12]
    out_t = out.ap().rearrange("(n p) d -> n p d", p=128)    # [16,128,D]

    bw = [None]   # bwork pool, created after xwp release

    with tc.tile_pool(name="persist", bufs=1) as persist, \
         tc.tile_pool(name="qkvp", bufs=1) as qkvp, \
         tc.tile_pool(name="rawp", bufs=1) as rawp, \
         tc.tile_pool(name="sqp", bufs=1) as sqp, \
         tc.tile_pool(name="normp", bufs=2) as normp, \
         tc.tile_pool(name="ofp", bufs=3) as ofp, \
         tc.tile_pool(name="bigps", bufs=2, space="PSUM") as bigps, \
         tc.tile_pool(name="rps", bufs=2, space="PSUM") as rps, \
         tc.tile_pool(name="kps", bufs=2, space="PSUM") as kps, \
         tc.tile_pool(name="tps", bufs=2, space="PSUM") as tps:

        # ---------------- loads ----------------
        xwp = tc.alloc_tile_pool(name="xwp", bufs=1)
        cons = persist.tile([128, 1152], f16, name="cons", tag="cons")
        nc.sync.dma_start(cons[:], consts.ap())
        ident = cons[:, 0:128]        # I
        m_su2 = cons[:, 128:384]      # [+1 a<b] twice (head-pair masks)
        m_sl2 = cons[:, 384:640]      # [+1 a>b] twice
        m_R02 = cons[:, 640:896]      # [I - strict-upper] twice
        m_tri2 = cons[:, 896:1152]    # [+1 a<=b] twice
        ones_col = cons[:, 1023:1024]  # last col of triuI mask == all ones

        bias6 = persist.tile([128, 1], fp32, name="bias6", tag="bias6")
        nc.vector.memset(bias6[:], 1e-6)

        cwt = []
        for ct in range(4):
            t_ = persist.tile([128, 3 * CONV_K], fp32, name=f"cw{ct}",
                              tag=f"cw{ct}")
            nc.sync.dma_start(t_[:], cw_t[ct])
            cwt.append(t_)

        xt = []
        for kt in range(KT):
            t_ = xwp.tile([128, T], f16, name=f"xt{kt}", tag=f"xt{kt}")
            nc.sync.dma_start(t_[:], xT_t[kt])
            xt.append(t_)
        ws = {}
        for nm in ("q", "k", "v"):
            ws[nm] = []
            for kt in range(KT):
                t_ = xwp.tile([128, CG], f16, name=f"w{nm}{kt}",
                              tag=f"w{nm}{kt}")
                nc.sync.dma_start(t_[:], w_t[nm][kt])
                ws[nm].append(t_)
        wlist = [ws["q"], ws["k"], ws["v"]]
        wo_s = []
        for ct in range(4):
            t_ = persist.tile([128, D], f16, name=f"wos{ct}", tag=f"wos{ct}")
            nc.sync.dma_start(t_[:], wo_t[ct])
            wo_s.append(t_)

        # ---------------- persistent working tensors ----------------
        # qkh[ct]: [q | k] over time; vh[ct]: v; oTp[h]: output^T pair layout
        qkh = [qkvp.tile([128, 2 * T], f16, name=f"qkh{ct}", tag=f"qkh{ct}")
               for ct in range(4)]
        vh = [qkvp.tile([128, T], f16, name=f"vh{ct}", tag=f"vh{ct}")
              for ct in range(4)]
        oTp = [qkvp.tile([128, 2 * T], f16, name=f"oTp{h}", tag=f"oTp{h}")
               for h in range(2)]
        raw = [rawp.tile([128, HALF + PAD], f16, name=f"raw{i}", tag=f"raw{i}")
               for i in range(12)]
        for i in range(12):
            nc.gpsimd.memset(raw[i][:, 0:PAD], 0.0)

        s_sb = [None, None]

        # diag(conv weight) tiles for the v-projection conv-as-matmul
        dgv = []
        for ct in range(4):
            row = []
            for i in range(CONV_K):
                d_ = persist.tile([128, 128], f16, name=f"dgv{ct}{i}",
                                  tag=f"dgv{ct}{i}")
                nc.vector.tensor_scalar_mul(
                    d_[:], ident, cwt[ct][:, 2 * CONV_K + i:2 * CONV_K + i + 1])
                row.append(d_)
            dgv.append(row)

        # ============ phase A emission (per half) ============
        def emit_proj_block(half, nb):
            """Projection matmuls + psum->raw copies for one 512-token block."""
            gb = 2 * half + nb
            for ti in range(3):
                for ct in range(4):
                    idx = ti * 4 + ct
                    pp = bigps.tile([128, TOKB], fp32, name=f"pp{gb}{idx}",
                                    tag="big")
                    for kt in range(KT):
                        nc.tensor.matmul(
                            pp[:], wlist[ti][kt][:, ct * 128:(ct + 1) * 128],
                            xt[kt][:, gb * TOKB:(gb + 1) * TOKB],
                            start=(kt == 0), stop=(kt == KT - 1))
                    dst = raw[idx][:, PAD + nb * TOKB:PAD + (nb + 1) * TOKB]
                    if idx % 2 == 0:
                        nc.scalar.copy(dst, pp[:])
                    else:
                        nc.vector.tensor_copy(dst, pp[:])

        def _conv_dst(half, ti, ct):
            t0 = half * HALF
            if ti == 0:
                return qkh[ct][:, t0:t0 + HALF]
            if ti == 1:
                return qkh[ct][:, T + t0:T + t0 + HALF]
            return vh[ct][:, t0:t0 + HALF]

        def emit_conv_taps(half, ti, ct):
            """Causal conv (4 taps) for one (proj, ct) over one half.
            Silu is emitted separately to batch activation-table usage.
            v tiles (ti==2) run the conv on the PE as accumulating
            diag-weight matmuls, with Silu consuming the psum directly."""
            idx = ti * 4 + ct
            dst = _conv_dst(half, ti, ct)
            if ti == 2:
                for nb in range(2):
                    cv = bigps.tile([128, TOKB], fp32, name=f"cv{half}{ct}{nb}",
                                    tag="big")
                    for i in range(CONV_K):
                        nc.tensor.matmul(
                            cv[:], dgv[ct][i],
                            raw[idx][:, 1 + i + nb * TOKB:
                                     1 + i + nb * TOKB + TOKB],
                            start=(i == 0), stop=(i == CONV_K - 1))
                    nc.scalar.activation(
                        dst[:, nb * TOKB:(nb + 1) * TOKB], cv[:], AF.Silu)
                if half == 0:
                    nc.gpsimd.tensor_copy(raw[idx][:, 0:PAD],
                                          raw[idx][:, HALF:HALF + PAD])
                return
            w0 = cwt[ct][:, ti * CONV_K:ti * CONV_K + 1]
            nc.scalar.activation(dst, raw[idx][:, 1:1 + HALF], AF.Copy,
                                 scale=w0)
            tta = sqp.tile([128, HALF], f16, name=f"cta{half}{idx}", tag="cta",
                           bufs=3)
            ttb = sqp.tile([128, HALF], f16, name=f"ctb{half}{idx}", tag="ctb",
                           bufs=3)
            w1 = cwt[ct][:, ti * CONV_K + 1:ti * CONV_K + 2]
            w2 = cwt[ct][:, ti * CONV_K + 2:ti * CONV_K + 3]
            w3 = cwt[ct][:, ti * CONV_K + 3:ti * CONV_K + 4]
            nc.vector.tensor_scalar_mul(tta[:], raw[idx][:, 2:2 + HALF], w1)
            nc.vector.tensor_scalar_mul(ttb[:], raw[idx][:, 3:3 + HALF], w2)
            nc.vector.tensor_add(tta[:], tta[:], ttb[:])
            nc.vector.tensor_scalar_mul(ttb[:], raw[idx][:, 4:4 + HALF], w3)
            nc.vector.tensor_add(dst, dst, tta[:])
            nc.vector.tensor_add(dst, dst, ttb[:])
            # boundary carry for next half (tokens 1020..1023 -> cols 0..3)
            if half == 0:
                nc.gpsimd.tensor_copy(raw[idx][:, 0:PAD],
                                      raw[idx][:, HALF:HALF + PAD])

        def emit_silu(half, ti, ct):
            if ti == 2:
                return
            dst = _conv_dst(half, ti, ct)
            nc.scalar.activation(dst, dst, AF.Silu)

        sq_q = {}   # (half, ct) -> [128, HALF] q^2 tiles for the RMS bias
        def emit_norms(half):
            """k l2norm (+ sq_q tiles) for one half."""
            t0 = half * HALF
            etn = nc.gpsimd if half == 0 else nc.vector
            for ct in range(4):
                t_ = sqp.tile([128, HALF], f16, name=f"sqq{half}{ct}",
                              tag=f"sqq{ct}", bufs=2)
                qs = qkh[ct][:, t0:t0 + HALF]
                etn.tensor_mul(t_[:], qs, qs)
                sq_q[(half, ct)] = t_
            for head in range(2):
                sqk = []
                for i in range(2):
                    ct = 2 * head + i
                    t_ = sqp.tile([128, HALF], f16, name=f"sqk{half}{ct}",
                                  tag="cta", bufs=3)
                    ks = qkh[ct][:, T + t0:T + t0 + HALF]
                    etn.tensor_mul(t_[:], ks, ks)
                    sqk.append(t_)
                bcf = normp.tile([128, HALF], fp32, name=f"bcf{half}{head}",
                                 tag="bcf", bufs=1)
                for nb in range(2):
                    prow = bigps.tile([1, TOKB], fp32,
                                      name=f"pr{half}{head}{nb}", tag="big")
                    for i in range(2):
                        nc.tensor.matmul(prow[:], ones_col,
                                         sqk[i][:, nb * TOKB:(nb + 1) * TOKB],
                                         start=(i == 0), stop=(i == 1))
                    rowb = normp.tile([1, TOKB], fp32,
                                      name=f"rb{half}{head}{nb}", tag="rowb",
                                      bufs=3)
                    nc.scalar.copy(rowb[:], prow[:])
                    nc.gpsimd.partition_broadcast(
                        bcf[:, nb * TOKB:(nb + 1) * TOKB], rowb[:])
                nc.scalar.activation(bcf[:], bcf[:], AF.Sqrt,
                                     bias=bias6[:, 0:1])
                nc.vector.reciprocal(bcf[:], bcf[:])
                bcb = normp.tile([128, HALF], f16, name=f"bcb{half}{head}",
                                 tag="bcb")
                etn.tensor_copy(bcb[:], bcf[:])
                for i in range(2):
                    ct = 2 * head + i
                    ks = qkh[ct][:, T + t0:T + t0 + HALF]
                    etn.tensor_mul(ks, ks, bcb[:])

        # ============ phase B emission: software-pipelined stages ============
        # PSUM rings (bank-granular, 8 banks):
        #   bigps x2: pp/prow (phase A), zw, pf
        #   rps  x2: rp [P2 pair | PT2 pair], dac [acc pair]
        #   kps  x2: qps, pkkq, pks, ksu0, ksu1, po
        #   tps  x2: kvt (f16 x4), wot (WT + oT, f16 x4)
        # Iteration k emits chunk k's precompute (R doubling etc.) with chunk
        # k-1's chain/output stages spliced between the R levels, so every
        # engine has ready work queued during the R ping-pong latencies.
        ST = {}

        def st_pre(ch):
            t0 = ch * C
            half = ch // 8
            st = ST[ch] = {}
            kvt = tps.tile([128, 1024], f16, name=f"kvt{ch}", tag="tps")
            qps_t = kps.tile([128, 2], fp32, name=f"qps{ch}", tag="kps")
            pkkq = kps.tile([128, 512], fp32, name=f"pkkq{ch}", tag="kps")
            if ch % 2 == 0:
                # pair-level quad state: [c0h0 | c0h1 | c1h0 | c1h1]
                qst = ST[ch]
                qst["rpP"] = rps.tile([128, 512], fp32, name=f"rpP{ch}",
                                      tag="rps")
                qst["rpPT"] = rps.tile([128, 512], fp32, name=f"rpPT{ch}",
                                       tag="rps")
                qst["Bq"] = bw[0].tile([128, 512], f16, name=f"Bq{ch}",
                                       tag="Bp", bufs=2)
                qst["Aq"] = bw[0].tile([128, 512], f16, name=f"Aq{ch}",
                                       tag="Ap", bufs=2)
                qst["R0q"] = bw[0].tile([128, 512], f16, name=f"R0q{ch}",
                                        tag="Rp", bufs=3)
            else:
                qst = ST[ch - 1]
            qo = 256 * (ch % 2)
            Bp = qst["Bq"][:, qo:qo + 256]
            Ap = qst["Aq"][:, qo:qo + 256]
            R0p = qst["R0q"][:, qo:qo + 256]
            rhs_kv = [None, None]
            Pat = [None, None]
            for h in range(2):
                ct0 = 2 * h
                for srcv in range(2):  # 0: v, 1: k
                    for i in range(2):
                        if srcv == 0:
                            ap = vh[ct0 + i][:, t0:t0 + C]
                        else:
                            ap = qkh[ct0 + i][:, T + t0:T + t0 + C]
                        o0 = 512 * h + 256 * srcv + 128 * i
                        nc.tensor.transpose(kvt[:, o0:o0 + 128], ap, ident)
                rkv = bw[0].tile([128, 512], f16, name=f"rkv{ch}{h}", tag="rkv",
                                 bufs=4)
                nc.scalar.copy(rkv[:], kvt[:, 512 * h:512 * (h + 1)])
                rhs_kv[h] = rkv
                pk = pkkq[:, 256 * h:256 * (h + 1)]
                for i in range(2):
                    qk2 = qkh[ct0 + i].rearrange(
                        "p (n t) -> p n t", n=2)[:, :, t0:t0 + C]
                    nc.tensor.matmul(pk, qkh[ct0 + i][:, T + t0:T + t0 + C],
                                     qk2, start=(i == 0), stop=(i == 1))
                qps = qps_t[:, h:h + 1]
                for i in range(2):
                    nc.tensor.matmul(qps, sq_q[(half, ct0 + i)][
                        :, t0 - half * HALF:t0 - half * HALF + C],
                        ones_col, start=(h == 0 and i == 0), stop=(i == 1),
                        skip_group_check=True)
            qbp = bw[0].tile([128, 2], fp32, name=f"qb{ch}", tag="qb", bufs=4)
            nc.scalar.activation(qbp[:], qps_t[:], AF.Copy, scale=QBS)
            # head-paired mask ops ([h0|h1] strided reads of pkkq)
            pkk2 = pkkq.rearrange("p (h c) -> p h c", h=2)[:, :, 128:256]
            pkq2 = pkkq.rearrange("p (h c) -> p h c", h=2)[:, :, 0:128]
            B2 = Bp.rearrange("p (h c) -> p h c", h=2)
            A2_ = Ap.rearrange("p (h c) -> p h c", h=2)
            M2 = m_su2.rearrange("p (h c) -> p h c", h=2)
            nc.vector.tensor_mul(B2, pkk2, M2)
            nc.vector.tensor_mul(A2_, pkk2,
                                 m_sl2.rearrange("p (h c) -> p h c", h=2))
            for h in range(2):
                hs = slice(128 * h, 128 * (h + 1))
                nc.vector.tensor_sub(R0p[:, hs], ident, Bp[:, hs])
            Patp = bw[0].tile([128, 256], f16, name=f"Pat{ch}", tag="Pat",
                              bufs=4)
            nc.vector.tensor_mul(Patp.rearrange("p (h c) -> p h c", h=2),
                                 pkq2, m_tri2.rearrange("p (h c) -> p h c", h=2))
            Pat = [Patp[:, 0:128], Patp[:, 128:256]]
            st.update(rhs_kv=rhs_kv, Pat=Pat, qb=[qbp[:, 0:1], qbp[:, 1:2]])
            if ch % 2 == 1:
                qst.update(RT=qst["R0q"], Pm=qst["Bq"], PTm=qst["Aq"])

        def st_rlvl(ch, lvl):
            # quad level over the pair (ch is the even chunk)
            qst = ST[ch]
            rpP, rpPT = qst["rpP"], qst["rpPT"]
            RT, Pm, PTm = qst["RT"], qst["Pm"], qst["PTm"]
            for j in range(4):
                js = slice(128 * j, 128 * (j + 1))
                if lvl < NLVL - 1:
                    nc.tensor.matmul(rpP[:, js], PTm[:, js], Pm[:, js],
                                     start=True, stop=True,
                                     skip_group_check=True)
                nc.tensor.matmul(rpPT[:, js], Pm[:, js], PTm[:, js],
                                 start=True, stop=True, skip_group_check=True)
            PTn = bw[0].tile([128, 512], f16, name=f"ptn{ch}{lvl}", tag="PT",
                             bufs=3)
            nc.vector.tensor_copy(PTn[:], rpPT[:])
            if lvl < NLVL - 1:
                Pn = bw[0].tile([128, 512], f16, name=f"pn{ch}{lvl}", tag="P",
                                bufs=3)
                nc.scalar.copy(Pn[:], rpP[:])
            else:
                Pn = None
            for j in range(4):
                js = slice(128 * j, 128 * (j + 1))
                nc.tensor.matmul(rpP[:, js], PTn[:, js], RT[:, js],
                                 start=True, stop=True, skip_group_check=True)
            RTn = bw[0].tile([128, 512], f16, name=f"rt{ch}{lvl}", tag="Rp",
                             bufs=3)
            nc.vector.tensor_add(RTn[:], RT[:], rpP[:])
            qst.update(RT=RTn, Pm=Pn, PTm=PTn)

        def st_zw(ch):
            st = ST[ch]
            RTq = ST[ch - ch % 2]["RT"]
            RT = RTq[:, 256 * (ch % 2):256 * (ch % 2) + 256]
            rhs_kv = st["rhs_kv"]
            zwp = bw[0].tile([128, 1024], f16, name=f"zwp{ch}", tag="zw")
            wtp = tps.tile([128, 512], f16, name=f"wtp{ch}", tag="tps")
            for h in range(2):
                zw = bigps.tile([128, 512], fp32, name=f"zw{ch}{h}", tag="big")
                nc.tensor.matmul(zw[:], RT[:, 128 * h:128 * (h + 1)],
                                 rhs_kv[h][:], start=True, stop=True)
                if h == 0:
                    nc.scalar.copy(zwp[:, 0:512], zw[:])
                else:
                    nc.scalar.copy(zwp[:, 512:1024], zw[:])
            for h in range(2):
                for i in range(2):
                    nc.tensor.transpose(
                        wtp[:, 256 * h + 128 * i:256 * h + 128 * (i + 1)],
                        zwp[:, 512 * h + 256 + 128 * i:
                            512 * h + 256 + 128 * (i + 1)],
                        ident)
            wts = bw[0].tile([128, 512], f16, name=f"wts{ch}", tag="wt")
            nc.scalar.copy(wts[:], wtp[:])
            st.update(zwp=zwp, wts=wts)

        def st_chain1(ch):
            st = ST[ch]
            zwp, wts = st["zwp"], st["wts"]
            s_prev = [s_sb[0], s_sb[1]]
            up = bw[0].tile([128, 512], f16, name=f"up{ch}", tag="u", bufs=4)
            zsel = zwp.rearrange("p (n c) -> p n c", n=4)[:, 0::2, :]
            if ch == 0:
                nc.vector.tensor_copy(
                    up.rearrange("p (n c) -> p n c", n=2), zsel)
            else:
                pks_t = kps.tile([128, 512], fp32, name=f"pks{ch}", tag="kps")
                for h in range(2):
                    pks = pks_t[:, 256 * h:256 * (h + 1)]
                    for i in range(2):
                        nc.tensor.matmul(
                            pks,
                            wts[:, 256 * h + 128 * i:256 * h + 128 * (i + 1)],
                            s_prev[h][:, i * 256:(i + 1) * 256],
                            start=(i == 0), stop=(i == 1))
                nc.vector.tensor_sub(
                    up.rearrange("p (n c) -> p n c", n=2), zsel, pks_t[:])
            st.update(up=up, s_prev=s_prev)

        def st_chain2(ch):
            st = ST[ch]
            up, s_prev, rhs_kv = st["up"], st["s_prev"], st["rhs_kv"]
            for h in range(2):
                ksu = kps.tile([128, 512], fp32, name=f"ksu{ch}{h}", tag="kps")
                for i in range(2):
                    # start once per bank: start=True marks the WHOLE 2KB
                    # bank pending-zero; i=1's start=False write overwrites
                    # its still-pending half (init semantics).
                    nc.tensor.matmul(
                        ksu[:, i * 256:(i + 1) * 256],
                        rhs_kv[h][:, 256 + 128 * i:256 + 128 * (i + 1)],
                        up[:, 256 * h:256 * (h + 1)],
                        start=(i == 0), stop=True, skip_group_check=True)
                s_n = bw[0].tile([128, 512], f16, name=f"ssb{ch}{h}",
                                 tag="ssb", bufs=4)
                if ch == 0:
                    if h == 0:
                        nc.vector.tensor_copy(s_n[:], ksu[:])
                    else:
                        nc.scalar.copy(s_n[:], ksu[:])
                else:
                    nc.vector.tensor_add(s_n[:], s_prev[h][:], ksu[:])
                s_sb[h] = s_n

        def st_o1(ch):
            st = ST[ch]
            t0 = ch * C
            up, s_prev, Pat, qb = st["up"], st["s_prev"], st["Pat"], st["qb"]
            po_t = rps.tile([128, 512], fp32, name=f"po{ch}", tag="rps")
            onrm = bw[0].tile([128, 512], f16, name=f"onrm{ch}", tag="onrm")
            for h in range(2):
                ct0 = 2 * h
                po = po_t[:, 256 * h:256 * (h + 1)]
                if ch == 0:
                    nc.tensor.matmul(po, Pat[h],
                                     up[:, 256 * h:256 * (h + 1)],
                                     start=(h == 0), stop=True,
                                     skip_group_check=True)
                else:
                    for i in range(2):
                        nc.tensor.matmul(po, qkh[ct0 + i][:, t0:t0 + C],
                                         s_prev[h][:, i * 256:(i + 1) * 256],
                                         start=(h == 0 and i == 0), stop=False,
                                         skip_group_check=True)
                    nc.tensor.matmul(po, Pat[h],
                                     up[:, 256 * h:256 * (h + 1)],
                                     start=False, stop=True,
                                     skip_group_check=True)
                osq = bw[0].tile([128, 256], f16, name=f"osq{ch}{h}",
                                 tag="osq")
                ossq = bw[0].tile([128, 1], fp32, name=f"ossq{ch}{h}",
                                  tag="ossq", bufs=4)
                nc.scalar.activation(osq[:], po, AF.Square, accum_out=ossq[:])
                orsq = bw[0].tile([128, 1], fp32, name=f"orsq{ch}{h}",
                                  tag="orsq", bufs=4)
                nc.scalar.activation(orsq[:], ossq[:], AF.Sqrt,
                                     bias=qb[h], scale=1.0 / DH)
                nc.vector.reciprocal(orsq[:], orsq[:])
                nc.vector.tensor_scalar_mul(onrm[:, 256 * h:256 * (h + 1)],
                                            po, orsq[:])
            st.update(onrm=onrm)

        def st_o2c(ch):
            st = ST[ch]
            t0 = ch * C
            onrm = st["onrm"]
            otp = tps.tile([128, 512], f16, name=f"otp{ch}", tag="tps")
            for h in range(2):
                for i in range(2):
                    nc.tensor.transpose(
                        otp[:, 256 * h + 128 * i:256 * h + 128 * (i + 1)],
                        onrm[:, 256 * h + i * 128:256 * h + (i + 1) * 128],
                        ident)
                dstp = oTp[h].rearrange("p (n t) -> p n t",
                                        n=2)[:, :, t0:t0 + C]
                srcp = otp[:, 256 * h:256 * (h + 1)].rearrange(
                    "p (n t) -> p n t", n=2)
                nc.scalar.activation(dstp, srcp, AF.Copy)
            for hf in range(2):
                pf = bigps.tile([128, 512], fp32, name=f"pf{ch}{hf}",
                                tag="big")
                for ct in range(4):
                    h, i = divmod(ct, 2)
                    nc.tensor.matmul(
                        pf[:], oTp[h][:, i * T + t0:i * T + t0 + C],
                        wo_s[ct][:, hf * 512:(hf + 1) * 512],
                        start=(ct == 0), stop=(ct == 3))
                of = ofp.tile([128, 512], f16, name=f"of{ch}{hf}", tag="of")
                nc.scalar.copy(of[:], pf[:])
                nc.sync.dma_start(out_t[ch][:, hf * 512:(hf + 1) * 512], of[:])
            ST.pop(ch, None)

        # ============ top-level emission order ============
        emit_proj_block(0, 0)
        emit_proj_block(0, 1)
        for ti in range(3):
            for ct in range(4):
                emit_conv_taps(0, ti, ct)
                emit_silu(0, ti, ct)
        emit_norms(0)
        emit_proj_block(1, 0)
        emit_proj_block(1, 1)
        # projection inputs are dead now; reuse their SBUF for phase B work
        xwp.release()
        bw[0] = tc.alloc_tile_pool(name="bwork", bufs=3)
        for h in range(2):
            t_ = bw[0].tile([128, 512], f16, name=f"ssb{h}_init", tag="ssb",
                            bufs=4)
            nc.vector.memset(t_[:], 0.0)
            s_sb[h] = t_

        def _silus_norms():
            for ti in range(3):
                for ct in range(4):
                    emit_silu(1, ti, ct)
            emit_norms(1)

        a1 = [
            lambda: [emit_conv_taps(1, 0, ct) for ct in range(4)],
            lambda: [emit_conv_taps(1, 1, ct) for ct in range(4)],
            lambda: [emit_conv_taps(1, 2, ct) for ct in range(4)],
            _silus_norms,
        ]

        for it in range(NCHUNK // 2 + 1):
            c0, c1 = 2 * it, 2 * it + 1
            p0, p1 = c0 - 2, c1 - 2
            pre = c0 < NCHUNK
            if pre:
                st_pre(c0)
                st_pre(c1)
                st_rlvl(c0, 0)
            if p0 >= 0:
                st_chain1(p0)
            if pre:
                st_rlvl(c0, 1)
            if p0 >= 0:
                st_chain2(p0)
                st_chain1(p1)
            if pre:
                st_rlvl(c0, 2)
            if p0 >= 0:
                st_chain2(p1)
                st_o1(p0)
            if pre:
                st_rlvl(c0, 3)
            if p0 >= 0:
                st_o1(p1)
                st_o2c(p0)
                st_o2c(p1)
            if pre:
                st_zw(c0)
                st_zw(c1)
            if it < len(a1):
                a1[it]()
        bw[0].release()


LP_NP = np.float16


def _make_consts():
    ii = np.arange(128)
    ident = np.eye(128, dtype=np.float32)
    m_su = (ii[:, None] < ii[None, :]).astype(np.float32)
    m_sl = (ii[:, None] > ii[None, :]).astype(np.float32)
    m_R0 = ident - m_su
    m_triuI = (ii[:, None] <= ii[None, :]).astype(np.float32)
    return np.concatenate([ident, m_su, m_su, m_sl, m_sl, m_R0, m_R0,
                           m_triuI, m_triuI], axis=1).astype(LP_NP)


def _get_compiled():
    if "nc" not in _CACHE:
        _CACHE["nc"] = _build_bass()
    return _CACHE["nc"]


def kernel(hidden_states, Wq, Wk, Wv, conv_wq, conv_wk, conv_wv, onorm_w, Wo):
    from concourse.bass_utils import run_bass_kernel_spmd

    hidden_states = np.asarray(hidden_states, np.float32)
    Wq = np.asarray(Wq, np.float32)
    Wk = np.asarray(Wk, np.float32)
    Wv = np.asarray(Wv, np.float32)
    Wo = np.asarray(Wo, np.float32)
    conv_wq = np.asarray(conv_wq, np.float32)
    conv_wk = np.asarray(conv_wk, np.float32)
    conv_wv = np.asarray(conv_wv, np.float32)
    onorm_w = np.asarray(onorm_w, np.float32)

    consts = _make_consts()
    Wo_eff = (Wo * np.tile(onorm_w, H)[:, None]).astype(LP_NP)

    in_maps = []
    for core in range(NCORES):
        b, g = divmod(core, 2)
        cols = slice(CG * g, CG * (g + 1))
        cwf = np.concatenate([conv_wq[cols], conv_wk[cols], conv_wv[cols]],
                             axis=1)
        in_maps.append({
            "xT": np.ascontiguousarray(hidden_states[b].T).astype(LP_NP),
            "wq": np.ascontiguousarray(Wq[:, cols]).astype(LP_NP),
            "wk": np.ascontiguousarray(Wk[:, cols]).astype(LP_NP),
            "wv": np.ascontiguousarray(Wv[:, cols]).astype(LP_NP),
            "wo": np.ascontiguousarray(Wo_eff[cols, :]),
            "cw": np.ascontiguousarray(cwf.reshape(4, 128, 3 * CONV_K)),
            "consts": consts,
        })

    nc = _get_compiled()
    res = run_bass_kernel_spmd(nc, in_maps, core_ids=list(range(NCORES)),
                               **_CACHE.get("run_kwargs", {}))
    _CACHE["last_results"] = res
    out = np.zeros((B, T, D), np.float32)
    for core in range(NCORES):
        out[core // 2] += res.results[core]["out"].astype(np.float32)
    return out


# revision 44
# speedup vs baseline: 1.0462x; 1.0211x over previous
"""DeltaNet forward kernel for 8 Trainium2 NeuronCores (v3).

Problem (hardcoded): hidden_states [B=4, T=2048, D=1024], H=4 heads, Dh=256,
causal depthwise conv K=4 + silu on q/k/v projections, q/k l2-normalized per
head (q scaled Dh^-0.5), delta-rule recurrence over T, per-head RMSNorm,
merge heads, out = o @ Wo.

Sharding: core c -> batch c//2, head group c%2 (512 projection columns).
Each core computes a partial product against its 512 rows of Wo; the host
sums the two partials per batch.

Design vs baseline:
- q l2norm folded into the output RMSNorm bias:
  out = o_raw / sqrt(mean(o_raw^2) + 256*EPS*|q_raw|^2) (exact up to 2.56e-9).
- Chunked delta rule (C=128) with the chunk inverse computed densely:
  RT = (I+B)^-1 (B = strict upper of K K^T) via 4-level Neumann doubling
  using the transposed-pair trick (track P=B^2^k and P^T together so every
  matmul has its stationary operand pre-transposed). Exponents <= 31;
  validated 1e-4 (f64) / ~3e-3 (fp16) against the exact recurrence.
- Per chunk precompute [Z|W] = R [V|K]; the S-dependent critical path is
  only: pks = W S -> u = Z - pks -> S += K^T u -> copy S (4 hops).
- Both heads interleaved per chunk; head-paired elementwise ops in the
  R chain; phase A runs in 2 halves with half 1 spliced between chunks
  0..7; the output projection streams per 128-token chunk.
- fp16 everywhere (fp8 tested: quantization error does not average down
  for random-sign dot products -> ~4% output error, over budget).
- Activation-table discipline: Copy/Square are in every act table; Silu
  and Sqrt never share one. All Silus batched so tables load ~4x total.
"""

import numpy as np

B, T, D = 4, 2048, 1024
H = 4
DH = D // H          # 256
CONV_K = 4
EPS = 1e-5
NCORES = 8
CG = 512             # columns per core (2 heads)
C = 128              # recurrence chunk length
NCHUNK = T // C      # 16
PAD = 4              # leading zero pad for causal conv
TOKB = 512           # projection token block (psum width)
HALF = 1024          # conv/norm granularity
NLVL = 4             # doubling levels (exponents <= 2^(NLVL+1)-1 = 31)
KT = 8               # contraction tiles for projections
QBS = float(EPS * DH)   # 2.56e-3: q-sumsq scale folded into RMS bias

_CACHE = {}
DBG = False

# tap0 engine per (ti, ct) flat index 0..11: 1 = Act (Copy*scale), 0 = DVE
CONV_ENG = [1] * 12


def _build_bass():
    import concourse.bass as bass  # noqa: F401
    import concourse.bacc as bacc
    import concourse.mybir as mybir
    import concourse.tile as tile

    dt = mybir.dt
    nc = bacc.Bacc("TRN2", target_bir_lowering=False, debug=False)

    xT = nc.dram_tensor("xT", [D, T], dt.float16, kind="ExternalInput")
    wq = nc.dram_tensor("wq", [D, CG], dt.float16, kind="ExternalInput")
    wk = nc.dram_tensor("wk", [D, CG], dt.float16, kind="ExternalInput")
    wv = nc.dram_tensor("wv", [D, CG], dt.float16, kind="ExternalInput")
    wo = nc.dram_tensor("wo", [CG, D], dt.float16, kind="ExternalInput")
    cw = nc.dram_tensor("cw", [4, 128, 3 * CONV_K], dt.float32,
                        kind="ExternalInput")
    consts = nc.dram_tensor("consts", [128, 1152], dt.float16,
                            kind="ExternalInput")
    out = nc.dram_tensor("out", [T, D], dt.float16, kind="ExternalOutput")
    dbg = nc.dram_tensor("dbg", [128, 5120], dt.float32,
                         kind="ExternalOutput") if DBG else None

    with tile.TileContext(nc) as tc:
        _body(nc, tc, mybir, xT, wq, wk, wv, wo, cw, consts, out, dbg)

    nc.compile()
    return nc


def _body(nc, tc, mybir, xT, wq, wk, wv, wo, cw, consts, out, dbg=None):
    dt = mybir.dt
    AF = mybir.ActivationFunctionType
    ALU = mybir.AluOpType
    fp32 = dt.float32
    f16 = dt.float16

    xT_t = xT.ap().rearrange("(n p) t -> n p t", p=128)      # [8,128,T]
    w_t = {"q": wq.ap().rearrange("(n p) c -> n p c", p=128),
           "k": wk.ap().rearrange("(n p) c -> n p c", p=128),
           "v": wv.ap().rearrange("(n p) c -> n p c", p=128)}
    wo_t = wo.ap().rearrange("(n p) c -> n p c", p=128)      # [4,128,D]
    cw_t = cw.ap()                                           # [4,128,12]
    out_t = out.ap().rearrange("(n p) d -> n p d", p=128)    # [16,128,D]

    bw = [None]   # bwork pool, created after xwp release

    with tc.tile_pool(name="persist", bufs=1) as persist, \
         tc.tile_pool(name="qkvp", bufs=1) as qkvp, \
         tc.tile_pool(name="rawp", bufs=1) as rawp, \
         tc.tile_pool(name="sqp", bufs=1) as sqp, \
         tc.tile_pool(name="normp", bufs=2) as normp, \
         tc.tile_pool(name="ofp", bufs=3) as ofp, \
         tc.tile_pool(name="bigps", bufs=2, space="PSUM") as bigps, \
         tc.tile_pool(name="rps", bufs=2, space="PSUM") as rps, \
         tc.tile_pool(name="kps", bufs=2, space="PSUM") as kps, \
         tc.tile_pool(name="tps", bufs=2, space="PSUM") as tps:

        # ---------------- loads ----------------
        xwp = tc.alloc_tile_pool(name="xwp", bufs=1)
        cons = persist.tile([128, 1152], f16, name="cons", tag="cons")
        nc.sync.dma_start(cons[:], consts.ap())
        ident = cons[:, 0:128]        # I
        m_su2 = cons[:, 128:384]      # [+1 a<b] twice (head-pair masks)
        m_sl2 = cons[:, 384:640]      # [+1 a>b] twice
        m_R02 = cons[:, 640:896]      # [I - strict-upper] twice
        m_tri2 = cons[:, 896:1152]    # [+1 a<=b] twice
        ones_col = cons[:, 1023:1024]  # last col of triuI mask == all ones

        bias6 = persist.tile([128, 1], fp32, name="bias6", tag="bias6")
        nc.vector.memset(bias6[:], 1e-6)

        cwt = []
        for ct in range(4):
            t_ = persist.tile([128, 3 * CONV_K], fp32, name=f"cw{ct}",
                              tag=f"cw{ct}")
            nc.sync.dma_start(t_[:], cw_t[ct])
            cwt.append(t_)

        xt = []
        for kt in range(KT):
            t_ = xwp.tile([128, T], f16, name=f"xt{kt}", tag=f"xt{kt}")
            nc.sync.dma_start(t_[:], xT_t[kt])
            xt.append(t_)
        ws = {}
        for nm in ("q", "k", "v"):
            ws[nm] = []
            for kt in range(KT):
                t_ = xwp.tile([128, CG], f16, name=f"w{nm}{kt}",
                              tag=f"w{nm}{kt}")
                nc.sync.dma_start(t_[:], w_t[nm][kt])
                ws[nm].append(t_)
        wlist = [ws["q"], ws["k"], ws["v"]]
        wo_s = []
        for ct in range(4):
            t_ = persist.tile([128, D], f16, name=f"wos{ct}", tag=f"wos{ct}")
            nc.sync.dma_start(t_[:], wo_t[ct])
            wo_s.append(t_)

        # ---------------- persistent working tensors ----------------
        # qkh[ct]: [q | k] over time; vh[ct]: v; oTp[h]: output^T pair layout
        qkh = [qkvp.tile([128, 2 * T], f16, name=f"qkh{ct}", tag=f"qkh{ct}")
               for ct in range(4)]
        vh = [qkvp.tile([128, T], f16, name=f"vh{ct}", tag=f"vh{ct}")
              for ct in range(4)]
        oTp = [qkvp.tile([128, 2 * T], f16, name=f"oTp{h}", tag=f"oTp{h}")
               for h in range(2)]
        raw = [rawp.tile([128, HALF + PAD], f16, name=f"raw{i}", tag=f"raw{i}")
               for i in range(12)]
        for i in range(12):
            nc.gpsimd.memset(raw[i][:, 0:PAD], 0.0)

        s_sb = [None, None]

        # diag(conv weight) tiles for the v-projection conv-as-matmul
        dgv = []
        for ct in range(4):
            row = []
            for i in range(CONV_K):
                d_ = persist.tile([128, 128], f16, name=f"dgv{ct}{i}",
                                  tag=f"dgv{ct}{i}")
                nc.vector.tensor_scalar_mul(
                    d_[:], ident, cwt[ct][:, 2 * CONV_K + i:2 * CONV_K + i + 1])
                row.append(d_)
            dgv.append(row)

        # ============ phase A emission (per half) ============
        def emit_proj_block(half, nb):
            """Projection matmuls + psum->raw copies for one 512-token block."""
            gb = 2 * half + nb
            for ti in range(3):
                for ct in range(4):
                    idx = ti * 4 + ct
                    pp = bigps.tile([128, TOKB], fp32, name=f"pp{gb}{idx}",
                                    tag="big")
                    for kt in range(KT):
                        nc.tensor.matmul(
                            pp[:], wlist[ti][kt][:, ct * 128:(ct + 1) * 128],
                            xt[kt][:, gb * TOKB:(gb + 1) * TOKB],
                            start=(kt == 0), stop=(kt == KT - 1))
                    dst = raw[idx][:, PAD + nb * TOKB:PAD + (nb + 1) * TOKB]
                    if idx % 2 == 0:
                        nc.scalar.copy(dst, pp[:])
                    else:
                        nc.vector.tensor_copy(dst, pp[:])

        def _conv_dst(half, ti, ct):
            t0 = half * HALF
            if ti == 0:
                return qkh[ct][:, t0:t0 + HALF]
            if ti == 1:
                return qkh[ct][:, T + t0:T + t0 + HALF]
            return vh[ct][:, t0:t0 + HALF]

        def emit_conv_taps(half, ti, ct):
            """Causal conv (4 taps) for one (proj, ct) over one half.
            Silu is emitted separately to batch activation-table usage.
            v tiles (ti==2) run the conv on the PE as accumulating
            diag-weight matmuls, with Silu consuming the psum directly."""
            idx = ti * 4 + ct
            dst = _conv_dst(half, ti, ct)
            if ti == 2:
                for nb in range(2):
                    cv = bigps.tile([128, TOKB], fp32, name=f"cv{half}{ct}{nb}",
                                    tag="big")
                    for i in range(CONV_K):
                        nc.tensor.matmul(
                            cv[:], dgv[ct][i],
                            raw[idx][:, 1 + i + nb * TOKB:
                                     1 + i + nb * TOKB + TOKB],
                            start=(i == 0), stop=(i == CONV_K - 1))
                    nc.scalar.activation(
                        dst[:, nb * TOKB:(nb + 1) * TOKB], cv[:], AF.Silu)
                if half == 0:
                    nc.gpsimd.tensor_copy(raw[idx][:, 0:PAD],
                                          raw[idx][:, HALF:HALF + PAD])
                return
            w0 = cwt[ct][:, ti * CONV_K:ti * CONV_K + 1]
            nc.scalar.activation(dst, raw[idx][:, 1:1 + HALF], AF.Copy,
                                 scale=w0)
            tta = sqp.tile([128, HALF], f16, name=f"cta{half}{idx}", tag="cta",
                           bufs=3)
            ttb = sqp.tile([128, HALF], f16, name=f"ctb{half}{idx}", tag="ctb",
                           bufs=3)
            w1 = cwt[ct][:, ti * CONV_K + 1:ti * CONV_K + 2]
            w2 = cwt[ct][:, ti * CONV_K + 2:ti * CONV_K + 3]
            w3 = cwt[ct][:, ti * CONV_K + 3:ti * CONV_K + 4]
            nc.vector.tensor_scalar_mul(tta[:], raw[idx][:, 2:2 + HALF], w1)
            nc.vector.tensor_scalar_mul(ttb[:], raw[idx][:, 3:3 + HALF], w2)
            nc.vector.tensor_add(tta[:], tta[:], ttb[:])
            nc.vector.tensor_scalar_mul(ttb[:], raw[idx][:, 4:4 + HALF], w3)
            nc.vector.tensor_add(dst, dst, tta[:])
            nc.vector.tensor_add(dst, dst, ttb[:])
            # boundary carry for next half (tokens 1020..1023 -> cols 0..3)
            if half == 0:
                nc.gpsimd.tensor_copy(raw[idx][:, 0:PAD],
                                      raw[idx][:, HALF:HALF + PAD])

        def emit_silu(half, ti, ct):
            if ti == 2:
                return
            dst = _conv_dst(half, ti, ct)
            nc.scalar.activation(dst, dst, AF.Silu)

        sq_q = {}   # (half, ct) -> [128, HALF] q^2 tiles for the RMS bias
        def emit_norms(half):
            """k l2norm (+ sq_q tiles) for one half."""
            t0 = half * HALF
            etn = nc.gpsimd if half == 0 else nc.vector
            for ct in range(4):
                t_ = sqp.tile([128, HALF], f16, name=f"sqq{half}{ct}",
                              tag=f"sqq{ct}", bufs=2)
                qs = qkh[ct][:, t0:t0 + HALF]
                etn.tensor_mul(t_[:], qs, qs)
                sq_q[(half, ct)] = t_
            for head in range(2):
                sqk = []
                for i in range(2):
                    ct = 2 * head + i
                    t_ = sqp.tile([128, HALF], f16, name=f"sqk{half}{ct}",
                                  tag="cta", bufs=3)
                    ks = qkh[ct][:, T + t0:T + t0 + HALF]
                    etn.tensor_mul(t_[:], ks, ks)
                    sqk.append(t_)
                bcf = normp.tile([128, HALF], fp32, name=f"bcf{half}{head}",
                                 tag="bcf", bufs=1)
                for nb in range(2):
                    prow = bigps.tile([1, TOKB], fp32,
                                      name=f"pr{half}{head}{nb}", tag="big")
                    for i in range(2):
                        nc.tensor.matmul(prow[:], ones_col,
                                         sqk[i][:, nb * TOKB:(nb + 1) * TOKB],
                                         start=(i == 0), stop=(i == 1))
                    rowb = normp.tile([1, TOKB], fp32,
                                      name=f"rb{half}{head}{nb}", tag="rowb",
                                      bufs=3)
                    nc.scalar.copy(rowb[:], prow[:])
                    nc.gpsimd.partition_broadcast(
                        bcf[:, nb * TOKB:(nb + 1) * TOKB], rowb[:])
                nc.scalar.activation(bcf[:], bcf[:], AF.Sqrt,
                                     bias=bias6[:, 0:1])
                nc.vector.reciprocal(bcf[:], bcf[:])
                bcb = normp.tile([128, HALF], f16, name=f"bcb{half}{head}",
                                 tag="bcb")
                etn.tensor_copy(bcb[:], bcf[:])
                for i in range(2):
                    ct = 2 * head + i
                    ks = qkh[ct][:, T + t0:T + t0 + HALF]
                    etn.tensor_mul(ks, ks, bcb[:])

        # ============ phase B emission: software-pipelined stages ============
        # PSUM rings (bank-granular, 8 banks):
        #   bigps x2: pp/prow (phase A), zw, pf
        #   rps  x2: rp [P2 pair | PT2 pair], dac [acc pair]
        #   kps  x2: qps, pkkq, pks, ksu0, ksu1, po
        #   tps  x2: kvt (f16 x4), wot (WT + oT, f16 x4)
        # Iteration k emits chunk k's precompute (R doubling etc.) with chunk
        # k-1's chain/output stages spliced between the R levels, so every
        # engine has ready work queued during the R ping-pong latencies.
        ST = {}

        def st_pre(ch):
            t0 = ch * C
            half = ch // 8
            st = ST[ch] = {}
            kvt = tps.tile([128, 1024], f16, name=f"kvt{ch}", tag="tps")
            qps_t = kps.tile([128, 2], fp32, name=f"qps{ch}", tag="kps")
            pkkq = kps.tile([128, 512], fp32, name=f"pkkq{ch}", tag="kps")
            if ch % 2 == 0:
                # pair-level quad state: [c0h0 | c0h1 | c1h0 | c1h1]
                qst = ST[ch]
                qst["rpP"] = rps.tile([128, 512], fp32, name=f"rpP{ch}",
                                      tag="rps")
                qst["rpPT"] = rps.tile([128, 512], fp32, name=f"rpPT{ch}",
                                       tag="rps")
                qst["Bq"] = bw[0].tile([128, 512], f16, name=f"Bq{ch}",
                                       tag="Bp", bufs=2)
                qst["Aq"] = bw[0].tile([128, 512], f16, name=f"Aq{ch}",
                                       tag="Ap", bufs=2)
                qst["R0q"] = bw[0].tile([128, 512], f16, name=f"R0q{ch}",
                                        tag="Rp", bufs=3)
            else:
                qst = ST[ch - 1]
            qo = 256 * (ch % 2)
            Bp = qst["Bq"][:, qo:qo + 256]
            Ap = qst["Aq"][:, qo:qo + 256]
            R0p = qst["R0q"][:, qo:qo + 256]
            rhs_kv = [None, None]
            Pat = [None, None]
            for h in range(2):
                ct0 = 2 * h
                for srcv in range(2):  # 0: v, 1: k
                    for i in range(2):
                        if srcv == 0:
                            ap = vh[ct0 + i][:, t0:t0 + C]
                        else:
                            ap = qkh[ct0 + i][:, T + t0:T + t0 + C]
                        o0 = 512 * h + 256 * srcv + 128 * i
                        nc.tensor.transpose(kvt[:, o0:o0 + 128], ap, ident)
                rkv = bw[0].tile([128, 512], f16, name=f"rkv{ch}{h}", tag="rkv",
                                 bufs=4)
                nc.scalar.copy(rkv[:], kvt[:, 512 * h:512 * (h + 1)])
                rhs_kv[h] = rkv
                pk = pkkq[:, 256 * h:256 * (h + 1)]
                for i in range(2):
                    qk2 = qkh[ct0 + i].rearrange(
                        "p (n t) -> p n t", n=2)[:, :, t0:t0 + C]
                    nc.tensor.matmul(pk, qkh[ct0 + i][:, T + t0:T + t0 + C],
                                     qk2, start=(i == 0), stop=(i == 1))
                qps = qps_t[:, h:h + 1]
                for i in range(2):
                    nc.tensor.matmul(qps, sq_q[(half, ct0 + i)][
                        :, t0 - half * HALF:t0 - half * HALF + C],
                        ones_col, start=(h == 0 and i == 0), stop=(i == 1),
                        skip_group_check=True)
            qbp = bw[0].tile([128, 2], fp32, name=f"qb{ch}", tag="qb", bufs=4)
            nc.scalar.activation(qbp[:], qps_t[:], AF.Copy, scale=QBS)
            # head-paired mask ops ([h0|h1] strided reads of pkkq)
            pkk2 = pkkq.rearrange("p (h c) -> p h c", h=2)[:, :, 128:256]
            pkq2 = pkkq.rearrange("p (h c) -> p h c", h=2)[:, :, 0:128]
            B2 = Bp.rearrange("p (h c) -> p h c", h=2)
            A2_ = Ap.rearrange("p (h c) -> p h c", h=2)
            M2 = m_su2.rearrange("p (h c) -> p h c", h=2)
            nc.vector.tensor_mul(B2, pkk2, M2)
            nc.vector.tensor_mul(A2_, pkk2,
                                 m_sl2.rearrange("p (h c) -> p h c", h=2))
            for h in range(2):
                hs = slice(128 * h, 128 * (h + 1))
                nc.vector.tensor_sub(R0p[:, hs], ident, Bp[:, hs])
            Patp = bw[0].tile([128, 256], f16, name=f"Pat{ch}", tag="Pat",
                              bufs=4)
            nc.vector.tensor_mul(Patp.rearrange("p (h c) -> p h c", h=2),
                                 pkq2, m_tri2.rearrange("p (h c) -> p h c", h=2))
            Pat = [Patp[:, 0:128], Patp[:, 128:256]]
            st.update(rhs_kv=rhs_kv, Pat=Pat, qb=[qbp[:, 0:1], qbp[:, 1:2]])
            if ch % 2 == 1:
                qst.update(RT=qst["R0q"], Pm=qst["Bq"], PTm=qst["Aq"])

        def st_rlvl(ch, lvl):
            # quad level over the pair (ch is the even chunk)
            qst = ST[ch]
            rpP, rpPT = qst["rpP"], qst["rpPT"]
            RT, Pm, PTm = qst["RT"], qst["Pm"], qst["PTm"]
            for j in range(4):
                js = slice(128 * j, 128 * (j + 1))
                if lvl < NLVL - 1:
                    nc.tensor.matmul(rpP[:, js], PTm[:, js], Pm[:, js],
                                     start=True, stop=True,
                                     skip_group_check=True)
                nc.tensor.matmul(rpPT[:, js], Pm[:, js], PTm[:, js],
                                 start=True, stop=True, skip_group_check=True)
            PTn = bw[0].tile([128, 512], f16, name=f"ptn{ch}{lvl}", tag="PT",
                             bufs=3)
            nc.vector.tensor_copy(PTn[:], rpPT[:])
            if lvl < NLVL - 1:
                Pn = bw[0].tile([128, 512], f16, name=f"pn{ch}{lvl}", tag="P",
                                bufs=3)
                nc.scalar.copy(Pn[:], rpP[:])
            else:
                Pn = None
            for j in range(4):
                js = slice(128 * j, 128 * (j + 1))
                nc.tensor.matmul(rpP[:, js], PTn[:, js], RT[:, js],
                                 start=True, stop=True, skip_group_check=True)
            RTn = bw[0].tile([128, 512], f16, name=f"rt{ch}{lvl}", tag="Rp",
                             bufs=3)
            nc.vector.tensor_add(RTn[:], RT[:], rpP[:])
            qst.update(RT=RTn, Pm=Pn, PTm=PTn)

        def st_zw(ch):
            st = ST[ch]
            RTq = ST[ch - ch % 2]["RT"]
            RT = RTq[:, 256 * (ch % 2):256 * (ch % 2) + 256]
            rhs_kv = st["rhs_kv"]
            zwp = bw[0].tile([128, 1024], f16, name=f"zwp{ch}", tag="zw")
            wtp = tps.tile([128, 512], f16, name=f"wtp{ch}", tag="tps")
            for h in range(2):
                zw = bigps.tile([128, 512], fp32, name=f"zw{ch}{h}", tag="big")
                nc.tensor.matmul(zw[:], RT[:, 128 * h:128 * (h + 1)],
                                 rhs_kv[h][:], start=True, stop=True)
                if h == 0:
                    nc.vector.tensor_copy(zwp[:, 0:512], zw[:])
                else:
                    nc.scalar.copy(zwp[:, 512:1024], zw[:])
            for h in range(2):
                for i in range(2):
                    nc.tensor.transpose(
                        wtp[:, 256 * h + 128 * i:256 * h + 128 * (i + 1)],
                        zwp[:, 512 * h + 256 + 128 * i:
                            512 * h + 256 + 128 * (i + 1)],
                        ident)
            wts = bw[0].tile([128, 512], f16, name=f"wts{ch}", tag="wt")
            nc.scalar.copy(wts[:], wtp[:])
            st.update(zwp=zwp, wts=wts)

        def st_chain1(ch):
            st = ST[ch]
            zwp, wts = st["zwp"], st["wts"]
            s_prev = [s_sb[0], s_sb[1]]
            up = bw[0].tile([128, 512], f16, name=f"up{ch}", tag="u", bufs=4)
            zsel = zwp.rearrange("p (n c) -> p n c", n=4)[:, 0::2, :]
            if ch == 0:
                nc.vector.tensor_copy(
                    up.rearrange("p (n c) -> p n c", n=2), zsel)
            else:
                pks_t = kps.tile([128, 512], fp32, name=f"pks{ch}", tag="kps")
                for h in range(2):
                    pks = pks_t[:, 256 * h:256 * (h + 1)]
                    for i in range(2):
                        nc.tensor.matmul(
                            pks,
                            wts[:, 256 * h + 128 * i:256 * h + 128 * (i + 1)],
                            s_prev[h][:, i * 256:(i + 1) * 256],
                            start=(i == 0), stop=(i == 1))
                nc.vector.tensor_sub(
                    up.rearrange("p (n c) -> p n c", n=2), zsel, pks_t[:])
            st.update(up=up, s_prev=s_prev)

        def st_chain2(ch):
            st = ST[ch]
            up, s_prev, rhs_kv = st["up"], st["s_prev"], st["rhs_kv"]
            for h in range(2):
                ksu = kps.tile([128, 512], fp32, name=f"ksu{ch}{h}", tag="kps")
                for i in range(2):
                    # start once per bank: start=True marks the WHOLE 2KB
                    # bank pending-zero; i=1's start=False write overwrites
                    # its still-pending half (init semantics).
                    nc.tensor.matmul(
                        ksu[:, i * 256:(i + 1) * 256],
                        rhs_kv[h][:, 256 + 128 * i:256 + 128 * (i + 1)],
                        up[:, 256 * h:256 * (h + 1)],
                        start=(i == 0), stop=True, skip_group_check=True)
                s_n = bw[0].tile([128, 512], f16, name=f"ssb{ch}{h}",
                                 tag="ssb", bufs=4)
                if ch == 0:
                    if h == 0:
                        nc.vector.tensor_copy(s_n[:], ksu[:])
                    else:
                        nc.scalar.copy(s_n[:], ksu[:])
                else:
                    nc.vector.tensor_add(s_n[:], s_prev[h][:], ksu[:])
                s_sb[h] = s_n

        def st_o1(ch):
            st = ST[ch]
            t0 = ch * C
            up, s_prev, Pat, qb = st["up"], st["s_prev"], st["Pat"], st["qb"]
            po_t = rps.tile([128, 512], fp32, name=f"po{ch}", tag="rps")
            onrm = bw[0].tile([128, 512], f16, name=f"onrm{ch}", tag="onrm")
            for h in range(2):
                ct0 = 2 * h
                po = po_t[:, 256 * h:256 * (h + 1)]
                if ch == 0:
                    nc.tensor.matmul(po, Pat[h],
                                     up[:, 256 * h:256 * (h + 1)],
                                     start=(h == 0), stop=True,
                                     skip_group_check=True)
                else:
                    for i in range(2):
                        nc.tensor.matmul(po, qkh[ct0 + i][:, t0:t0 + C],
                                         s_prev[h][:, i * 256:(i + 1) * 256],
                                         start=(h == 0 and i == 0), stop=False,
                                         skip_group_check=True)
                    nc.tensor.matmul(po, Pat[h],
                                     up[:, 256 * h:256 * (h + 1)],
                                     start=False, stop=True,
                                     skip_group_check=True)
                osq = bw[0].tile([128, 256], f16, name=f"osq{ch}{h}",
                                 tag="osq")
                ossq = bw[0].tile([128, 1], fp32, name=f"ossq{ch}{h}",
                                  tag="ossq", bufs=4)
                nc.scalar.activation(osq[:], po, AF.Square, accum_out=ossq[:])
                orsq = bw[0].tile([128, 1], fp32, name=f"orsq{ch}{h}",
                                  tag="orsq", bufs=4)
                nc.scalar.activation(orsq[:], ossq[:], AF.Sqrt,
                                     bias=qb[h], scale=1.0 / DH)
                nc.vector.reciprocal(orsq[:], orsq[:])
                nc.vector.tensor_scalar_mul(onrm[:, 256 * h:256 * (h + 1)],
                                            po, orsq[:])
            st.update(onrm=onrm)

        def st_o2c(ch):
            st = ST[ch]
            t0 = ch * C
            onrm = st["onrm"]
            otp = tps.tile([128, 512], f16, name=f"otp{ch}", tag="tps")
            for h in range(2):
                for i in range(2):
                    nc.tensor.transpose(
                        otp[:, 256 * h + 128 * i:256 * h + 128 * (i + 1)],
                        onrm[:, 256 * h + i * 128:256 * h + (i + 1) * 128],
                        ident)
                dstp = oTp[h].rearrange("p (n t) -> p n t",
                                        n=2)[:, :, t0:t0 + C]
                srcp = otp[:, 256 * h:256 * (h + 1)].rearrange(
                    "p (n t) -> p n t", n=2)
                nc.scalar.activation(dstp, srcp, AF.Copy)
            for hf in range(2):
                pf = bigps.tile([128, 512], fp32, name=f"pf{ch}{hf}",
                                tag="big")
                for ct in range(4):
                    h, i = divmod(ct, 2)
                    nc.tensor.matmul(
                        pf[:], oTp[h][:, i * T + t0:i * T + t0 + C],
                        wo_s[ct][:, hf * 512:(hf + 1) * 512],
                        start=(ct == 0), stop=(ct == 3))
                of = ofp.tile([128, 512], f16, name=f"of{ch}{hf}", tag="of")
                nc.scalar.copy(of[:], pf[:])
                nc.sync.dma_start(out_t[ch][:, hf * 512:(hf + 1) * 512], of[:])
            ST.pop(ch, None)

        # ============ top-level emission order ============
        emit_proj_block(0, 0)
        emit_proj_block(0, 1)
        for ti in range(3):
            for ct in range(4):
                emit_conv_taps(0, ti, ct)
                emit_silu(0, ti, ct)
        emit_norms(0)
        emit_proj_block(1, 0)
        emit_proj_block(1, 1)
        # projection inputs are dead now; reuse their SBUF for phase B work
        xwp.release()
        bw[0] = tc.alloc_tile_pool(name="bwork", bufs=3)
        for h in range(2):
            t_ = bw[0].tile([128, 512], f16, name=f"ssb{h}_init", tag="ssb",
                            bufs=4)
            nc.vector.memset(t_[:], 0.0)
            s_sb[h] = t_

        def _silus_norms():
            for ti in range(3):
                for ct in range(4):
                    emit_silu(1, ti, ct)
            emit_norms(1)

        a1 = [
            lambda: [emit_conv_taps(1, 0, ct) for ct in range(4)],
            lambda: [emit_conv_taps(1, 1, ct) for ct in range(4)],
            lambda: [emit_conv_taps(1, 2, ct) for ct in range(4)],
            _silus_norms,
        ]

        for it in range(NCHUNK // 2 + 1):
            c0, c1 = 2 * it, 2 * it + 1
            p0, p1 = c0 - 2, c1 - 2
            pre = c0 < NCHUNK
            if pre:
                st_pre(c0)
                st_pre(c1)
                st_rlvl(c0, 0)
            if p0 >= 0:
                st_chain1(p0)
            if pre:
                st_rlvl(c0, 1)
            if p0 >= 0:
                st_chain2(p0)
                st_chain1(p1)
            if pre:
                st_rlvl(c0, 2)
            if p0 >= 0:
                st_chain2(p1)
                st_o1(p0)
            if pre:
                st_rlvl(c0, 3)
            if p0 >= 0:
                st_o1(p1)
                st_o2c(p0)
                st_o2c(p1)
            if pre:
                st_zw(c0)
                st_zw(c1)
            if it < len(a1):
                a1[it]()
        bw[0].release()


LP_NP = np.float16


def _make_consts():
    ii = np.arange(128)
    ident = np.eye(128, dtype=np.float32)
    m_su = (ii[:, None] < ii[None, :]).astype(np.float32)
    m_sl = (ii[:, None] > ii[None, :]).astype(np.float32)
    m_R0 = ident - m_su
    m_triuI = (ii[:, None] <= ii[None, :]).astype(np.float32)
    return np.concatenate([ident, m_su, m_su, m_sl, m_sl, m_R0, m_R0,
                           m_triuI, m_triuI], axis=1).astype(LP_NP)


def _get_compiled():
    if "nc" not in _CACHE:
        _CACHE["nc"] = _build_bass()
    return _CACHE["nc"]


def kernel(hidden_states, Wq, Wk, Wv, conv_wq, conv_wk, conv_wv, onorm_w, Wo):
    from concourse.bass_utils import run_bass_kernel_spmd

    hidden_states = np.asarray(hidden_states, np.float32)
    Wq = np.asarray(Wq, np.float32)
    Wk = np.asarray(Wk, np.float32)
    Wv = np.asarray(Wv, np.float32)
    Wo = np.asarray(Wo, np.float32)
    conv_wq = np.asarray(conv_wq, np.float32)
    conv_wk = np.asarray(conv_wk, np.float32)
    conv_wv = np.asarray(conv_wv, np.float32)
    onorm_w = np.asarray(onorm_w, np.float32)

    consts = _make_consts()
    Wo_eff = (Wo * np.tile(onorm_w, H)[:, None]).astype(LP_NP)

    in_maps = []
    for core in range(NCORES):
        b, g = divmod(core, 2)
        cols = slice(CG * g, CG * (g + 1))
        cwf = np.concatenate([conv_wq[cols], conv_wk[cols], conv_wv[cols]],
                             axis=1)
        in_maps.append({
            "xT": np.ascontiguousarray(hidden_states[b].T).astype(LP_NP),
            "wq": np.ascontiguousarray(Wq[:, cols]).astype(LP_NP),
            "wk": np.ascontiguousarray(Wk[:, cols]).astype(LP_NP),
            "wv": np.ascontiguousarray(Wv[:, cols]).astype(LP_NP),
            "wo": np.ascontiguousarray(Wo_eff[cols, :]),
            "cw": np.ascontiguousarray(cwf.reshape(4, 128, 3 * CONV_K)),
            "consts": consts,
        })

    nc = _get_compiled()
    res = run_bass_kernel_spmd(nc, in_maps, core_ids=list(range(NCORES)),
                               **_CACHE.get("run_kwargs", {}))
    _CACHE["last_results"] = res
    out = np.zeros((B, T, D), np.float32)
    for core in range(NCORES):
        out[core // 2] += res.results[core]["out"].astype(np.float32)
    return out


# revision 46
# speedup vs baseline: 1.0722x; 1.0249x over previous
"""DeltaNet forward kernel for 8 Trainium2 NeuronCores (v3).

Problem (hardcoded): hidden_states [B=4, T=2048, D=1024], H=4 heads, Dh=256,
causal depthwise conv K=4 + silu on q/k/v projections, q/k l2-normalized per
head (q scaled Dh^-0.5), delta-rule recurrence over T, per-head RMSNorm,
merge heads, out = o @ Wo.

Sharding: core c -> batch c//2, head group c%2 (512 projection columns).
Each core computes a partial product against its 512 rows of Wo; the host
sums the two partials per batch.

Design vs baseline:
- q l2norm folded into the output RMSNorm bias:
  out = o_raw / sqrt(mean(o_raw^2) + 256*EPS*|q_raw|^2) (exact up to 2.56e-9).
- Chunked delta rule (C=128) with the chunk inverse computed densely:
  RT = (I+B)^-1 (B = strict upper of K K^T) via 4-level Neumann doubling
  using the transposed-pair trick (track P=B^2^k and P^T together so every
  matmul has its stationary operand pre-transposed). Exponents <= 31;
  validated 1e-4 (f64) / ~3e-3 (fp16) against the exact recurrence.
- Per chunk precompute [Z|W] = R [V|K]; the S-dependent critical path is
  only: pks = W S -> u = Z - pks -> S += K^T u -> copy S (4 hops).
- Both heads interleaved per chunk; head-paired elementwise ops in the
  R chain; phase A runs in 2 halves with half 1 spliced between chunks
  0..7; the output projection streams per 128-token chunk.
- fp16 everywhere (fp8 tested: quantization error does not average down
  for random-sign dot products -> ~4% output error, over budget).
- Activation-table discipline: Copy/Square are in every act table; Silu
  and Sqrt never share one. All Silus batched so tables load ~4x total.
"""

import numpy as np

B, T, D = 4, 2048, 1024
H = 4
DH = D // H          # 256
CONV_K = 4
EPS = 1e-5
NCORES = 8
CG = 512             # columns per core (2 heads)
C = 128              # recurrence chunk length
NCHUNK = T // C      # 16
PAD = 4              # leading zero pad for causal conv
TOKB = 512           # projection token block (psum width)
HALF = 1024          # conv/norm granularity
NLVL = 4             # doubling levels (exponents <= 2^(NLVL+1)-1 = 31)
KT = 8               # contraction tiles for projections
QBS = float(EPS * DH)   # 2.56e-3: q-sumsq scale folded into RMS bias

_CACHE = {}
DBG = False

# tap0 engine per (ti, ct) flat index 0..11: 1 = Act (Copy*scale), 0 = DVE
CONV_ENG = [1] * 12


def _build_bass():
    import concourse.bass as bass  # noqa: F401
    import concourse.bacc as bacc
    import concourse.mybir as mybir
    import concourse.tile as tile

    dt = mybir.dt
    nc = bacc.Bacc("TRN2", target_bir_lowering=False, debug=False)

    xT = nc.dram_tensor("xT", [D, T], dt.float16, kind="ExternalInput")
    wq = nc.dram_tensor("wq", [D, CG], dt.float16, kind="ExternalInput")
    wk = nc.dram_tensor("wk", [D, CG], dt.float16, kind="ExternalInput")
    wv = nc.dram_tensor("wv", [D, CG], dt.float16, kind="ExternalInput")
    wo = nc.dram_tensor("wo", [CG, D], dt.float16, kind="ExternalInput")
    cw = nc.dram_tensor("cw", [4, 128, 3 * CONV_K], dt.float32,
                        kind="ExternalInput")
    consts = nc.dram_tensor("consts", [128, 1152], dt.float16,
                            kind="ExternalInput")
    out = nc.dram_tensor("out", [T, D], dt.float16, kind="ExternalOutput")
    dbg = nc.dram_tensor("dbg", [128, 5120], dt.float32,
                         kind="ExternalOutput") if DBG else None

    with tile.TileContext(nc) as tc:
        _body(nc, tc, mybir, xT, wq, wk, wv, wo, cw, consts, out, dbg)

    nc.compile()
    return nc


def _body(nc, tc, mybir, xT, wq, wk, wv, wo, cw, consts, out, dbg=None):
    dt = mybir.dt
    AF = mybir.ActivationFunctionType
    ALU = mybir.AluOpType
    fp32 = dt.float32
    f16 = dt.float16

    xT_t = xT.ap().rearrange("(n p) t -> n p t", p=128)      # [8,128,T]
    w_t = {"q": wq.ap().rearrange("(n p) c -> n p c", p=128),
           "k": wk.ap().rearrange("(n p) c -> n p c", p=128),
           "v": wv.ap().rearrange("(n p) c -> n p c", p=128)}
    wo_t = wo.ap().rearrange("(n p) c -> n p c", p=128)      # [4,128,D]
    cw_t = cw.ap()                                           # [4,128,12]
    out_t = out.ap().rearrange("(n p) d -> n p d", p=128)    # [16,128,D]

    bw = [None]   # bwork pool, created after xwp release

    with tc.tile_pool(name="persist", bufs=1) as persist, \
         tc.tile_pool(name="qkvp", bufs=1) as qkvp, \
         tc.tile_pool(name="rawp", bufs=1) as rawp, \
         tc.tile_pool(name="sqp", bufs=1) as sqp, \
         tc.tile_pool(name="normp", bufs=2) as normp, \
         tc.tile_pool(name="ofp", bufs=3) as ofp, \
         tc.tile_pool(name="bigps", bufs=2, space="PSUM") as bigps, \
         tc.tile_pool(name="rps", bufs=2, space="PSUM") as rps, \
         tc.tile_pool(name="kps", bufs=2, space="PSUM") as kps, \
         tc.tile_pool(name="tps", bufs=2, space="PSUM") as tps:

        # ---------------- loads ----------------
        xwp = tc.alloc_tile_pool(name="xwp", bufs=1)
        cons = persist.tile([128, 1152], f16, name="cons", tag="cons")
        nc.sync.dma_start(cons[:], consts.ap())
        ident = cons[:, 0:128]        # I
        m_su2 = cons[:, 128:384]      # [+1 a<b] twice (head-pair masks)
        m_sl2 = cons[:, 384:640]      # [+1 a>b] twice
        m_R02 = cons[:, 640:896]      # [I - strict-upper] twice
        m_tri2 = cons[:, 896:1152]    # [+1 a<=b] twice
        ones_col = cons[:, 1023:1024]  # last col of triuI mask == all ones

        bias6 = persist.tile([128, 1], fp32, name="bias6", tag="bias6")
        nc.vector.memset(bias6[:], 1e-6)

        cwt = []
        for ct in range(4):
            t_ = persist.tile([128, 3 * CONV_K], fp32, name=f"cw{ct}",
                              tag=f"cw{ct}")
            nc.sync.dma_start(t_[:], cw_t[ct])
            cwt.append(t_)

        xt = []
        for kt in range(KT):
            t_ = xwp.tile([128, T], f16, name=f"xt{kt}", tag=f"xt{kt}")
            nc.sync.dma_start(t_[:], xT_t[kt])
            xt.append(t_)
        ws = {}
        for nm in ("q", "k", "v"):
            ws[nm] = []
            for kt in range(KT):
                t_ = xwp.tile([128, CG], f16, name=f"w{nm}{kt}",
                              tag=f"w{nm}{kt}")
                nc.sync.dma_start(t_[:], w_t[nm][kt])
                ws[nm].append(t_)
        wlist = [ws["q"], ws["k"], ws["v"]]
        wo_s = []
        for ct in range(4):
            t_ = persist.tile([128, D], f16, name=f"wos{ct}", tag=f"wos{ct}")
            nc.sync.dma_start(t_[:], wo_t[ct])
            wo_s.append(t_)

        # ---------------- persistent working tensors ----------------
        # qkh[ct]: [q | k] over time; vh[ct]: v; oTp[h]: output^T pair layout
        qkh = [qkvp.tile([128, 2 * T], f16, name=f"qkh{ct}", tag=f"qkh{ct}")
               for ct in range(4)]
        vh = [qkvp.tile([128, T], f16, name=f"vh{ct}", tag=f"vh{ct}")
              for ct in range(4)]
        oTp = [qkvp.tile([128, 2 * T], f16, name=f"oTp{h}", tag=f"oTp{h}")
               for h in range(2)]
        raw = [rawp.tile([128, HALF + PAD], f16, name=f"raw{i}", tag=f"raw{i}")
               for i in range(12)]
        for i in range(12):
            nc.gpsimd.memset(raw[i][:, 0:PAD], 0.0)

        s_sb = [None, None]

        # diag(conv weight) tiles for the v-projection conv-as-matmul
        dgv = []
        for ct in range(4):
            row = []
            for i in range(CONV_K):
                d_ = persist.tile([128, 128], f16, name=f"dgv{ct}{i}",
                                  tag=f"dgv{ct}{i}")
                nc.vector.tensor_scalar_mul(
                    d_[:], ident, cwt[ct][:, 2 * CONV_K + i:2 * CONV_K + i + 1])
                row.append(d_)
            dgv.append(row)

        # ============ phase A emission (per half) ============
        def emit_proj_block(half, nb):
            """Projection matmuls + psum->raw copies for one 512-token block."""
            gb = 2 * half + nb
            for ti in range(3):
                for ct in range(4):
                    idx = ti * 4 + ct
                    pp = bigps.tile([128, TOKB], fp32, name=f"pp{gb}{idx}",
                                    tag="big")
                    for kt in range(KT):
                        nc.tensor.matmul(
                            pp[:], wlist[ti][kt][:, ct * 128:(ct + 1) * 128],
                            xt[kt][:, gb * TOKB:(gb + 1) * TOKB],
                            start=(kt == 0), stop=(kt == KT - 1))
                    dst = raw[idx][:, PAD + nb * TOKB:PAD + (nb + 1) * TOKB]
                    if idx % 2 == 0:
                        nc.scalar.copy(dst, pp[:])
                    else:
                        nc.vector.tensor_copy(dst, pp[:])

        def _conv_dst(half, ti, ct):
            t0 = half * HALF
            if ti == 0:
                return qkh[ct][:, t0:t0 + HALF]
            if ti == 1:
                return qkh[ct][:, T + t0:T + t0 + HALF]
            return vh[ct][:, t0:t0 + HALF]

        def emit_conv_taps(half, ti, ct):
            """Causal conv (4 taps) for one (proj, ct) over one half.
            Silu is emitted separately to batch activation-table usage.
            v tiles (ti==2) run the conv on the PE as accumulating
            diag-weight matmuls, with Silu consuming the psum directly."""
            idx = ti * 4 + ct
            dst = _conv_dst(half, ti, ct)
            if ti == 2:
                for nb in range(2):
                    cv = bigps.tile([128, TOKB], fp32, name=f"cv{half}{ct}{nb}",
                                    tag="big")
                    for i in range(CONV_K):
                        nc.tensor.matmul(
                            cv[:], dgv[ct][i],
                            raw[idx][:, 1 + i + nb * TOKB:
                                     1 + i + nb * TOKB + TOKB],
                            start=(i == 0), stop=(i == CONV_K - 1))
                    nc.scalar.activation(
                        dst[:, nb * TOKB:(nb + 1) * TOKB], cv[:], AF.Silu)
                if half == 0:
                    nc.gpsimd.tensor_copy(raw[idx][:, 0:PAD],
                                          raw[idx][:, HALF:HALF + PAD])
                return
            w0 = cwt[ct][:, ti * CONV_K:ti * CONV_K + 1]
            nc.scalar.activation(dst, raw[idx][:, 1:1 + HALF], AF.Copy,
                                 scale=w0)
            tta = sqp.tile([128, HALF], f16, name=f"cta{half}{idx}", tag="cta",
                           bufs=3)
            ttb = sqp.tile([128, HALF], f16, name=f"ctb{half}{idx}", tag="ctb",
                           bufs=3)
            w1 = cwt[ct][:, ti * CONV_K + 1:ti * CONV_K + 2]
            w2 = cwt[ct][:, ti * CONV_K + 2:ti * CONV_K + 3]
            w3 = cwt[ct][:, ti * CONV_K + 3:ti * CONV_K + 4]
            nc.vector.tensor_scalar_mul(tta[:], raw[idx][:, 2:2 + HALF], w1)
            nc.vector.tensor_scalar_mul(ttb[:], raw[idx][:, 3:3 + HALF], w2)
            nc.vector.tensor_add(tta[:], tta[:], ttb[:])
            nc.vector.tensor_scalar_mul(ttb[:], raw[idx][:, 4:4 + HALF], w3)
            nc.vector.tensor_add(dst, dst, tta[:])
            nc.vector.tensor_add(dst, dst, ttb[:])
            # boundary carry for next half (tokens 1020..1023 -> cols 0..3)
            if half == 0:
                nc.gpsimd.tensor_copy(raw[idx][:, 0:PAD],
                                      raw[idx][:, HALF:HALF + PAD])

        def emit_silu(half, ti, ct):
            if ti == 2:
                return
            dst = _conv_dst(half, ti, ct)
            nc.scalar.activation(dst, dst, AF.Silu)

        sq_q = {}   # (half, ct) -> [128, HALF] q^2 tiles for the RMS bias
        def emit_norms(half):
            """k l2norm (+ sq_q tiles) for one half."""
            t0 = half * HALF
            etn = nc.gpsimd if half == 0 else nc.vector
            for ct in range(4):
                t_ = sqp.tile([128, HALF], f16, name=f"sqq{half}{ct}",
                              tag=f"sqq{ct}", bufs=2)
                qs = qkh[ct][:, t0:t0 + HALF]
                etn.tensor_mul(t_[:], qs, qs)
                sq_q[(half, ct)] = t_
            for head in range(2):
                sqk = []
                for i in range(2):
                    ct = 2 * head + i
                    t_ = sqp.tile([128, HALF], f16, name=f"sqk{half}{ct}",
                                  tag="cta", bufs=3)
                    ks = qkh[ct][:, T + t0:T + t0 + HALF]
                    etn.tensor_mul(t_[:], ks, ks)
                    sqk.append(t_)
                bcf = normp.tile([128, HALF], fp32, name=f"bcf{half}{head}",
                                 tag="bcf", bufs=1)
                for nb in range(2):
                    prow = bigps.tile([1, TOKB], fp32,
                                      name=f"pr{half}{head}{nb}", tag="big")
                    for i in range(2):
                        nc.tensor.matmul(prow[:], ones_col,
                                         sqk[i][:, nb * TOKB:(nb + 1) * TOKB],
                                         start=(i == 0), stop=(i == 1))
                    rowb = normp.tile([1, TOKB], fp32,
                                      name=f"rb{half}{head}{nb}", tag="rowb",
                                      bufs=3)
                    nc.scalar.copy(rowb[:], prow[:])
                    nc.gpsimd.partition_broadcast(
                        bcf[:, nb * TOKB:(nb + 1) * TOKB], rowb[:])
                nc.scalar.activation(bcf[:], bcf[:], AF.Sqrt,
                                     bias=bias6[:, 0:1])
                nc.vector.reciprocal(bcf[:], bcf[:])
                bcb = normp.tile([128, HALF], f16, name=f"bcb{half}{head}",
                                 tag="bcb")
                etn.tensor_copy(bcb[:], bcf[:])
                for i in range(2):
                    ct = 2 * head + i
                    ks = qkh[ct][:, T + t0:T + t0 + HALF]
                    etn.tensor_mul(ks, ks, bcb[:])

        # ============ phase B emission: software-pipelined stages ============
        # PSUM rings (bank-granular, 8 banks):
        #   bigps x2: pp/prow (phase A), zw, pf
        #   rps  x2: rp [P2 pair | PT2 pair], dac [acc pair]
        #   kps  x2: qps, pkkq, pks, ksu0, ksu1, po
        #   tps  x2: kvt (f16 x4), wot (WT + oT, f16 x4)
        # Iteration k emits chunk k's precompute (R doubling etc.) with chunk
        # k-1's chain/output stages spliced between the R levels, so every
        # engine has ready work queued during the R ping-pong latencies.
        ST = {}

        def st_pre(ch):
            t0 = ch * C
            half = ch // 8
            st = ST[ch] = {}
            kvt = tps.tile([128, 1024], f16, name=f"kvt{ch}", tag="tps")
            qps_t = kps.tile([128, 2], fp32, name=f"qps{ch}", tag="kps")
            pkkq = kps.tile([128, 512], fp32, name=f"pkkq{ch}", tag="kps")
            if ch % 2 == 0:
                # pair-level quad state: [c0h0 | c0h1 | c1h0 | c1h1]
                qst = ST[ch]
                qst["rpP"] = rps.tile([128, 512], fp32, name=f"rpP{ch}",
                                      tag="rps")
                qst["rpPT"] = rps.tile([128, 512], fp32, name=f"rpPT{ch}",
                                       tag="rps")
                qst["Bq"] = bw[0].tile([128, 512], f16, name=f"Bq{ch}",
                                       tag="Bp", bufs=2)
                qst["Aq"] = bw[0].tile([128, 512], f16, name=f"Aq{ch}",
                                       tag="Ap", bufs=2)
                qst["R0q"] = bw[0].tile([128, 512], f16, name=f"R0q{ch}",
                                        tag="Rp", bufs=3)
            else:
                qst = ST[ch - 1]
            qo = 256 * (ch % 2)
            Bp = qst["Bq"][:, qo:qo + 256]
            Ap = qst["Aq"][:, qo:qo + 256]
            R0p = qst["R0q"][:, qo:qo + 256]
            rhs_kv = [None, None]
            Pat = [None, None]
            for h in range(2):
                ct0 = 2 * h
                for srcv in range(2):  # 0: v, 1: k
                    for i in range(2):
                        if srcv == 0:
                            ap = vh[ct0 + i][:, t0:t0 + C]
                        else:
                            ap = qkh[ct0 + i][:, T + t0:T + t0 + C]
                        o0 = 512 * h + 256 * srcv + 128 * i
                        nc.tensor.transpose(kvt[:, o0:o0 + 128], ap, ident)
                rkv = bw[0].tile([128, 512], f16, name=f"rkv{ch}{h}", tag="rkv",
                                 bufs=4)
                nc.scalar.copy(rkv[:], kvt[:, 512 * h:512 * (h + 1)])
                rhs_kv[h] = rkv
                pk = pkkq[:, 256 * h:256 * (h + 1)]
                for i in range(2):
                    qk2 = qkh[ct0 + i].rearrange(
                        "p (n t) -> p n t", n=2)[:, :, t0:t0 + C]
                    nc.tensor.matmul(pk, qkh[ct0 + i][:, T + t0:T + t0 + C],
                                     qk2, start=(i == 0), stop=(i == 1))
                qps = qps_t[:, h:h + 1]
                for i in range(2):
                    nc.tensor.matmul(qps, sq_q[(half, ct0 + i)][
                        :, t0 - half * HALF:t0 - half * HALF + C],
                        ones_col, start=(h == 0 and i == 0), stop=(i == 1),
                        skip_group_check=True)
            qbp = bw[0].tile([128, 2], fp32, name=f"qb{ch}", tag="qb", bufs=4)
            nc.scalar.activation(qbp[:], qps_t[:], AF.Copy, scale=QBS)
            # head-paired mask ops ([h0|h1] strided reads of pkkq)
            pkk2 = pkkq.rearrange("p (h c) -> p h c", h=2)[:, :, 128:256]
            pkq2 = pkkq.rearrange("p (h c) -> p h c", h=2)[:, :, 0:128]
            B2 = Bp.rearrange("p (h c) -> p h c", h=2)
            A2_ = Ap.rearrange("p (h c) -> p h c", h=2)
            M2 = m_su2.rearrange("p (h c) -> p h c", h=2)
            nc.vector.tensor_mul(B2, pkk2, M2)
            nc.vector.tensor_mul(A2_, pkk2,
                                 m_sl2.rearrange("p (h c) -> p h c", h=2))
            for h in range(2):
                hs = slice(128 * h, 128 * (h + 1))
                nc.vector.tensor_sub(R0p[:, hs], ident, Bp[:, hs])
            Patp = bw[0].tile([128, 256], f16, name=f"Pat{ch}", tag="Pat",
                              bufs=4)
            nc.vector.tensor_mul(Patp.rearrange("p (h c) -> p h c", h=2),
                                 pkq2, m_tri2.rearrange("p (h c) -> p h c", h=2))
            Pat = [Patp[:, 0:128], Patp[:, 128:256]]
            st.update(rhs_kv=rhs_kv, Pat=Pat, qb=[qbp[:, 0:1], qbp[:, 1:2]])
            if ch % 2 == 1:
                qst.update(RT=qst["R0q"], Pm=qst["Bq"], PTm=qst["Aq"])

        def st_rlvl(ch, lvl):
            # quad level over the pair (ch is the even chunk)
            qst = ST[ch]
            rpP, rpPT = qst["rpP"], qst["rpPT"]
            RT, Pm, PTm = qst["RT"], qst["Pm"], qst["PTm"]
            for j in range(4):
                js = slice(128 * j, 128 * (j + 1))
                if lvl < NLVL - 1:
                    nc.tensor.matmul(rpP[:, js], PTm[:, js], Pm[:, js],
                                     start=True, stop=True,
                                     skip_group_check=True)
                nc.tensor.matmul(rpPT[:, js], Pm[:, js], PTm[:, js],
                                 start=True, stop=True, skip_group_check=True)
            PTn = bw[0].tile([128, 512], f16, name=f"ptn{ch}{lvl}", tag="PT",
                             bufs=3)
            nc.vector.tensor_copy(PTn[:], rpPT[:])
            if lvl < NLVL - 1:
                Pn = bw[0].tile([128, 512], f16, name=f"pn{ch}{lvl}", tag="P",
                                bufs=3)
                nc.scalar.copy(Pn[:], rpP[:])
            else:
                Pn = None
            for j in range(4):
                js = slice(128 * j, 128 * (j + 1))
                nc.tensor.matmul(rpP[:, js], PTn[:, js], RT[:, js],
                                 start=True, stop=True, skip_group_check=True)
            RTn = bw[0].tile([128, 512], f16, name=f"rt{ch}{lvl}", tag="Rp",
                             bufs=3)
            nc.vector.tensor_add(RTn[:], RT[:], rpP[:])
            qst.update(RT=RTn, Pm=Pn, PTm=PTn)

        def st_zw(ch):
            st = ST[ch]
            RTq = ST[ch - ch % 2]["RT"]
            RT = RTq[:, 256 * (ch % 2):256 * (ch % 2) + 256]
            rhs_kv = st["rhs_kv"]
            zwp = bw[0].tile([128, 1024], f16, name=f"zwp{ch}", tag="zw")
            wtp = tps.tile([128, 512], f16, name=f"wtp{ch}", tag="tps")
            for h in range(2):
                zw = bigps.tile([128, 512], fp32, name=f"zw{ch}{h}", tag="big")
                nc.tensor.matmul(zw[:], RT[:, 128 * h:128 * (h + 1)],
                                 rhs_kv[h][:], start=True, stop=True)
                if h == 0:
                    nc.vector.tensor_copy(zwp[:, 0:512], zw[:])
                else:
                    nc.scalar.copy(zwp[:, 512:1024], zw[:])
            for h in range(2):
                for i in range(2):
                    nc.tensor.transpose(
                        wtp[:, 256 * h + 128 * i:256 * h + 128 * (i + 1)],
                        zwp[:, 512 * h + 256 + 128 * i:
                            512 * h + 256 + 128 * (i + 1)],
                        ident)
            wts = bw[0].tile([128, 512], f16, name=f"wts{ch}", tag="wt")
            nc.scalar.copy(wts[:], wtp[:])
            st.update(zwp=zwp, wts=wts)

        def st_chain1(ch):
            st = ST[ch]
            zwp, wts = st["zwp"], st["wts"]
            s_prev = [s_sb[0], s_sb[1]]
            up = bw[0].tile([128, 512], f16, name=f"up{ch}", tag="u", bufs=4)
            zsel = zwp.rearrange("p (n c) -> p n c", n=4)[:, 0::2, :]
            if ch == 0:
                nc.vector.tensor_copy(
                    up.rearrange("p (n c) -> p n c", n=2), zsel)
            else:
                pks_t = kps.tile([128, 512], fp32, name=f"pks{ch}", tag="kps")
                for h in range(2):
                    pks = pks_t[:, 256 * h:256 * (h + 1)]
                    for i in range(2):
                        nc.tensor.matmul(
                            pks,
                            wts[:, 256 * h + 128 * i:256 * h + 128 * (i + 1)],
                            s_prev[h][:, i * 256:(i + 1) * 256],
                            start=(i == 0), stop=(i == 1))
                nc.vector.tensor_sub(
                    up.rearrange("p (n c) -> p n c", n=2), zsel, pks_t[:])
            st.update(up=up, s_prev=s_prev)

        def st_chain2(ch):
            st = ST[ch]
            up, s_prev, rhs_kv = st["up"], st["s_prev"], st["rhs_kv"]
            for h in range(2):
                ksu = kps.tile([128, 512], fp32, name=f"ksu{ch}{h}", tag="kps")
                for i in range(2):
                    # start once per bank: start=True marks the WHOLE 2KB
                    # bank pending-zero; i=1's start=False write overwrites
                    # its still-pending half (init semantics).
                    nc.tensor.matmul(
                        ksu[:, i * 256:(i + 1) * 256],
                        rhs_kv[h][:, 256 + 128 * i:256 + 128 * (i + 1)],
                        up[:, 256 * h:256 * (h + 1)],
                        start=(i == 0), stop=True, skip_group_check=True)
                s_n = bw[0].tile([128, 512], f16, name=f"ssb{ch}{h}",
                                 tag="ssb", bufs=4)
                if ch == 0:
                    if h == 0:
                        nc.vector.tensor_copy(s_n[:], ksu[:])
                    else:
                        nc.scalar.copy(s_n[:], ksu[:])
                else:
                    nc.vector.tensor_add(s_n[:], s_prev[h][:], ksu[:])
                s_sb[h] = s_n

        def st_o1(ch):
            st = ST[ch]
            t0 = ch * C
            up, s_prev, Pat, qb = st["up"], st["s_prev"], st["Pat"], st["qb"]
            po_t = rps.tile([128, 512], fp32, name=f"po{ch}", tag="rps")
            onrm = bw[0].tile([128, 512], f16, name=f"onrm{ch}", tag="onrm")
            for h in range(2):
                ct0 = 2 * h
                po = po_t[:, 256 * h:256 * (h + 1)]
                if ch == 0:
                    nc.tensor.matmul(po, Pat[h],
                                     up[:, 256 * h:256 * (h + 1)],
                                     start=(h == 0), stop=True,
                                     skip_group_check=True)
                else:
                    for i in range(2):
                        nc.tensor.matmul(po, qkh[ct0 + i][:, t0:t0 + C],
                                         s_prev[h][:, i * 256:(i + 1) * 256],
                                         start=(h == 0 and i == 0), stop=False,
                                         skip_group_check=True)
                    nc.tensor.matmul(po, Pat[h],
                                     up[:, 256 * h:256 * (h + 1)],
                                     start=False, stop=True,
                                     skip_group_check=True)
                osq = bw[0].tile([128, 256], f16, name=f"osq{ch}{h}",
                                 tag="osq")
                ossq = bw[0].tile([128, 1], fp32, name=f"ossq{ch}{h}",
                                  tag="ossq", bufs=4)
                nc.scalar.activation(osq[:], po, AF.Square, accum_out=ossq[:])
                orsq = bw[0].tile([128, 1], fp32, name=f"orsq{ch}{h}",
                                  tag="orsq", bufs=4)
                nc.scalar.activation(orsq[:], ossq[:], AF.Sqrt,
                                     bias=qb[h], scale=1.0 / DH)
                nc.vector.reciprocal(orsq[:], orsq[:])
                nc.vector.tensor_scalar_mul(onrm[:, 256 * h:256 * (h + 1)],
                                            po, orsq[:])
            st.update(onrm=onrm)

        def st_o2c(ch):
            st = ST[ch]
            t0 = ch * C
            onrm = st["onrm"]
            otp = tps.tile([128, 512], f16, name=f"otp{ch}", tag="tps")
            for h in range(2):
                for i in range(2):
                    nc.tensor.transpose(
                        otp[:, 256 * h + 128 * i:256 * h + 128 * (i + 1)],
                        onrm[:, 256 * h + i * 128:256 * h + (i + 1) * 128],
                        ident)
                dstp = oTp[h].rearrange("p (n t) -> p n t",
                                        n=2)[:, :, t0:t0 + C]
                srcp = otp[:, 256 * h:256 * (h + 1)].rearrange(
                    "p (n t) -> p n t", n=2)
                nc.scalar.activation(dstp, srcp, AF.Copy)
            for hf in range(2):
                pf = bigps.tile([128, 512], fp32, name=f"pf{ch}{hf}",
                                tag="big")
                for ct in range(4):
                    h, i = divmod(ct, 2)
                    nc.tensor.matmul(
                        pf[:], oTp[h][:, i * T + t0:i * T + t0 + C],
                        wo_s[ct][:, hf * 512:(hf + 1) * 512],
                        start=(ct == 0), stop=(ct == 3))
                of = ofp.tile([128, 512], f16, name=f"of{ch}{hf}", tag="of")
                nc.scalar.copy(of[:], pf[:])
                nc.sync.dma_start(out_t[ch][:, hf * 512:(hf + 1) * 512], of[:])
            ST.pop(ch, None)

        # ============ top-level emission order ============
        emit_proj_block(0, 0)
        emit_proj_block(0, 1)
        for ti in range(3):
            for ct in range(4):
                emit_conv_taps(0, ti, ct)
                emit_silu(0, ti, ct)
        emit_norms(0)
        emit_proj_block(1, 0)
        emit_proj_block(1, 1)
        # projection inputs are dead now; reuse their SBUF for phase B work
        xwp.release()
        bw[0] = tc.alloc_tile_pool(name="bwork", bufs=3)
        for h in range(2):
            t_ = bw[0].tile([128, 512], f16, name=f"ssb{h}_init", tag="ssb",
                            bufs=4)
            nc.vector.memset(t_[:], 0.0)
            s_sb[h] = t_

        def _silus_norms():
            for ti in range(3):
                for ct in range(4):
                    emit_silu(1, ti, ct)
            emit_norms(1)

        def _vtaps_silus_norms():
            # v taps carry fused Silus; keep them adjacent to the batched
            # q/k Silus so the act table switches Silu<->Sqrt only once here
            for ct in range(4):
                emit_conv_taps(1, 2, ct)
            _silus_norms()

        a1 = [
            lambda: [emit_conv_taps(1, 0, ct) for ct in range(4)],
            lambda: [emit_conv_taps(1, 1, ct) for ct in range(4)],
            _vtaps_silus_norms,
        ]

        for it in range(NCHUNK // 2 + 1):
            c0, c1 = 2 * it, 2 * it + 1
            p0, p1 = c0 - 2, c1 - 2
            pre = c0 < NCHUNK
            if pre:
                st_pre(c0)
                st_pre(c1)
                st_rlvl(c0, 0)
            if p0 >= 0:
                st_chain1(p0)
            if pre:
                st_rlvl(c0, 1)
            if p0 >= 0:
                st_chain2(p0)
                st_chain1(p1)
            if pre:
                st_rlvl(c0, 2)
            if p0 >= 0:
                st_chain2(p1)
                st_o1(p0)
            if pre:
                st_rlvl(c0, 3)
            if p0 >= 0:
                st_o1(p1)
                st_o2c(p0)
                st_o2c(p1)
            if pre:
                st_zw(c0)
                st_zw(c1)
            if it < len(a1):
                a1[it]()
        bw[0].release()


LP_NP = np.float16


def _make_consts():
    ii = np.arange(128)
    ident = np.eye(128, dtype=np.float32)
    m_su = (ii[:, None] < ii[None, :]).astype(np.float32)
    m_sl = (ii[:, None] > ii[None, :]).astype(np.float32)
    m_R0 = ident - m_su
    m_triuI = (ii[:, None] <= ii[None, :]).astype(np.float32)
    return np.concatenate([ident, m_su, m_su, m_sl, m_sl, m_R0, m_R0,
                           m_triuI, m_triuI], axis=1).astype(LP_NP)


def _get_compiled():
    if "nc" not in _CACHE:
        _CACHE["nc"] = _build_bass()
    return _CACHE["nc"]


def kernel(hidden_states, Wq, Wk, Wv, conv_wq, conv_wk, conv_wv, onorm_w, Wo):
    from concourse.bass_utils import run_bass_kernel_spmd

    hidden_states = np.asarray(hidden_states, np.float32)
    Wq = np.asarray(Wq, np.float32)
    Wk = np.asarray(Wk, np.float32)
    Wv = np.asarray(Wv, np.float32)
    Wo = np.asarray(Wo, np.float32)
    conv_wq = np.asarray(conv_wq, np.float32)
    conv_wk = np.asarray(conv_wk, np.float32)
    conv_wv = np.asarray(conv_wv, np.float32)
    onorm_w = np.asarray(onorm_w, np.float32)

    consts = _make_consts()
    Wo_eff = (Wo * np.tile(onorm_w, H)[:, None]).astype(LP_NP)

    in_maps = []
    for core in range(NCORES):
        b, g = divmod(core, 2)
        cols = slice(CG * g, CG * (g + 1))
        cwf = np.concatenate([conv_wq[cols], conv_wk[cols], conv_wv[cols]],
                             axis=1)
        in_maps.append({
            "xT": np.ascontiguousarray(hidden_states[b].T).astype(LP_NP),
            "wq": np.ascontiguousarray(Wq[:, cols]).astype(LP_NP),
            "wk": np.ascontiguousarray(Wk[:, cols]).astype(LP_NP),
            "wv": np.ascontiguousarray(Wv[:, cols]).astype(LP_NP),
            "wo": np.ascontiguousarray(Wo_eff[cols, :]),
            "cw": np.ascontiguousarray(cwf.reshape(4, 128, 3 * CONV_K)),
            "consts": consts,
        })

    nc = _get_compiled()
    res = run_bass_kernel_spmd(nc, in_maps, core_ids=list(range(NCORES)),
                               **_CACHE.get("run_kwargs", {}))
    _CACHE["last_results"] = res
    out = np.zeros((B, T, D), np.float32)
    for core in range(NCORES):
        out[core // 2] += res.results[core]["out"].astype(np.float32)
    return out
